# revision 1
# baseline (speedup 1.0000x reference)
"""Bass/Tile TRN2 kernel for nn_LAN_4320737100678 (dense transformer block).

Data-parallel over the batch axis across 8 NeuronCores (4 batches/core).
All activations are kept feature-major ([E, L] per batch) so that every
BatchNorm reduction and the softmax run along the free axis, and the BN
affine+ELU applications are single activation-engine passes with
per-partition scale/bias. The five BatchNorm moment sets are globalized
with four tiny in-kernel AllReduces (BN2+BN3 share one round).

The sliding-window "unfold" (W=5) is never materialized: stage 1 is
computed as 5 shifted matmuls accumulating into PSUM from a zero-padded
copy of m1^T.
"""

import os
import sys

sys.path.insert(0, "/opt/trn_rl_repo")

import numpy as np

import concourse.bass as bass
import concourse.tile as tile
from concourse import mybir
from concourse.bass_utils import run_bass_kernel_spmd
from concourse.masks import make_identity
from concourse.vector_clock import ScopedClock

N_CORES = 8
B, L, E, W = 32, 512, 512, 5
S = W // 2
P = 128
KC = E // P            # feature chunks of 128
B_LOC = B // N_CORES   # batches per core
EPS = 1e-3
F32 = mybir.dt.float32
AF = mybir.ActivationFunctionType
ALU = mybir.AluOpType
AX = mybir.AxisListType

# gpack column base offsets (each vector packed as [P, KC])
_G1, _B1, _G2, _B2, _G3, _B3, _G4, _B4, _G5, _B5 = (i * KC for i in range(10))

_MAX_CTRL_WAITS = 1


def _split_waits(nc, max_waits=_MAX_CTRL_WAITS):
    """walrus in this container encodes at most one sync-wait slot per
    instruction. Hoist extra waits onto same-engine NOPs inserted right
    before the owning instruction (same engine => executes first)."""
    for fn in nc.m.functions:
        for bb in fn.blocks:
            rebuilt = []
            changed = False
            for ins in bb.instructions:
                si = ins.sync_info
                if si is not None and len(si.on_wait) > max_waits:
                    waits = list(si.on_wait)
                    rest = waits[max_waits:]
                    for j in range(0, len(rest), max_waits):
                        nop = mybir.InstNoOp(
                            name=f"{ins.name}_wsplit{j}",
                            engine=ins.engine,
                            bass_nofuse=True,
                            sync_info=mybir.SyncInfo(
                                on_wait=rest[j : j + max_waits], on_update=[]
                            ),
                        )
                        rebuilt.append(nop)
                    ins.sync_info = mybir.SyncInfo(
                        on_wait=waits[:max_waits], on_update=list(si.on_update)
                    )
                    changed = True
                rebuilt.append(ins)
            if changed:
                bb.instructions = rebuilt


_CACHE = {}


def _build():
    if "nc" in _CACHE:
        return _CACHE["nc"]
    nc = bass.Bass("TRN2", target_bir_lowering=False, debug=False, num_devices=N_CORES)

    m1t_d = nc.dram_tensor("m1t", [B_LOC, E, L], F32, kind="ExternalInput")
    f_d = nc.dram_tensor("f", [W * E, E], F32, kind="ExternalInput")
    wq_d = nc.dram_tensor("wq", [E, E], F32, kind="ExternalInput")
    wk_d = nc.dram_tensor("wk", [E, E], F32, kind="ExternalInput")
    qbt_d = nc.dram_tensor("qbt", [E, L], F32, kind="ExternalInput")
    kbt_d = nc.dram_tensor("kbt", [E, L], F32, kind="ExternalInput")
    wbt_d = nc.dram_tensor("wbt", [L, L], F32, kind="ExternalInput")
    gp_d = nc.dram_tensor("gpack", [P, 10 * KC], F32, kind="ExternalInput")
    out_d = nc.dram_tensor("outt", [B_LOC, E, L], F32, kind="ExternalOutput")

    groups = [list(range(N_CORES))]

    from contextlib import ExitStack

    with tile.TileContext(nc) as tc:
        with (
            tc.tile_pool(name="const", bufs=1) as const,
            tc.tile_pool(name="bias", bufs=4) as biasp,
            tc.tile_pool(name="aff", bufs=44) as affp,
            tc.tile_pool(name="stats", bufs=24) as statp,
            tc.tile_pool(name="packs", bufs=8) as packp,
            tc.tile_pool(name="scr", bufs=16) as scr,
            tc.tile_pool(name="elu", bufs=3) as elup,
            tc.tile_pool(name="psum", bufs=4, space="PSUM") as psum,
            tc.tile_pool(name="psumT", bufs=4, space="PSUM") as psumT,
            tc.tile_pool(name="dram", bufs=8, space="DRAM") as dram,
        ):
            es_l = ExitStack()
            wtp = es_l.enter_context(tc.tile_pool(name="wT", bufs=B_LOC * KC))
            lp = es_l.enter_context(tc.tile_pool(name="l", bufs=B_LOC * KC))
            gp = const.tile([P, 10 * KC], F32, tag="gp")
            nc.sync.dma_start(out=gp[:], in_=gp_d[:])
            ident = const.tile([P, P], F32, tag="ident")
            make_identity(nc, ident[:])
            epst = const.tile([P, 1], F32, tag="eps")
            nc.vector.memset(epst[:], EPS)

            qbt_sb, kbt_sb, wbt_sb = {}, {}, {}
            for c in range(KC):
                t = biasp.tile([P, L], F32, tag="qbt")
                nc.sync.dma_start(out=t[:], in_=qbt_d[c * P : (c + 1) * P, :])
                qbt_sb[c] = t
                t = biasp.tile([P, L], F32, tag="kbt")
                nc.sync.dma_start(out=t[:], in_=kbt_d[c * P : (c + 1) * P, :])
                kbt_sb[c] = t
                t = biasp.tile([P, L], F32, tag="wbt")
                nc.sync.dma_start(out=t[:], in_=wbt_d[c * P : (c + 1) * P, :])
                wbt_sb[c] = t

            def stats_to_pack(stats_tiles, pack):
                """stats_tiles: per-chunk [P, B_LOC, 6] bn_stats rows.
                pack[:, c, 0] = local_mean/8, pack[:, c, 1] = local_E[x^2]/8."""
                for c, st in enumerate(stats_tiles):
                    mv = scr.tile([P, 2], F32, tag="scr")
                    nc.vector.bn_aggr(out=mv[:], in_=st[:])
                    sq = scr.tile([P, 1], F32, tag="scr")
                    nc.vector.tensor_mul(sq[:], mv[:, 0:1], mv[:, 0:1])
                    nc.vector.tensor_copy(pack[:, c, 0:1], mv[:, 0:1])
                    nc.vector.tensor_add(pack[:, c, 1:2], mv[:, 1:2], sq[:])
                nc.vector.tensor_scalar_mul(pack[:], pack[:], 1.0 / N_CORES)

            def allreduce(pack, width):
                cc_in = dram.tile([P, width], F32, tag="cc")
                cc_out = dram.tile([P, width], F32, tag="cc")
                nc.gpsimd.dma_start(out=cc_in[:], in_=pack[:])
                nc.gpsimd.collective_compute(
                    "AllReduce",
                    ALU.add,
                    replica_groups=groups,
                    ins=[cc_in.opt()],
                    outs=[cc_out.opt()],
                )
                g = packp.tile([P, width], F32, tag="g")
                nc.gpsimd.dma_start(out=g[:], in_=cc_out[:])
                return g

            def affines(g, gcol, bcol):
                """From allreduced [P, KC, 2] (mean, E[x^2]) compute per-chunk
                scale = gamma*rsqrt(var+eps), bias = beta - mean*scale."""
                sc_l, bi_l = [], []
                gv = g.rearrange("p (c two) -> p c two", two=2)
                for c in range(KC):
                    mean = gv[:, c, 0:1]
                    ex2 = gv[:, c, 1:2]
                    sq = scr.tile([P, 1], F32, tag="scr")
                    nc.vector.tensor_mul(sq[:], mean, mean)
                    var = scr.tile([P, 1], F32, tag="scr")
                    nc.vector.tensor_sub(var[:], ex2, sq[:])
                    sd = scr.tile([P, 1], F32, tag="scr")
                    nc.scalar.activation(out=sd[:], in_=var[:], func=AF.Sqrt, bias=epst[:])
                    rinv = scr.tile([P, 1], F32, tag="scr")
                    nc.vector.reciprocal(rinv[:], sd[:])
                    sc = affp.tile([P, 1], F32, tag="aff")
                    nc.vector.tensor_mul(sc[:], rinv[:], gp[:, gcol + c : gcol + c + 1])
                    tb = scr.tile([P, 1], F32, tag="scr")
                    nc.vector.tensor_mul(tb[:], mean, sc[:])
                    bi = affp.tile([P, 1], F32, tag="aff")
                    nc.vector.tensor_sub(bi[:], gp[:, bcol + c : bcol + c + 1], tb[:])
                    sc_l.append(sc)
                    bi_l.append(bi)
                return sc_l, bi_l

            def elu_apply(zt, sc, bi):
                """zt <- elu(zt*sc + bi) = relu(y) + (min(exp(y),1) - 1)."""
                e = elup.tile([P, L], F32, tag="elu_e")
                r = elup.tile([P, L], F32, tag="elu_r")
                nc.scalar.activation(out=e[:], in_=zt[:], func=AF.Exp, bias=bi[:], scale=sc[:])
                nc.scalar.activation(out=r[:], in_=zt[:], func=AF.Relu, bias=bi[:], scale=sc[:])
                nc.vector.tensor_scalar(
                    out=e[:], in0=e[:], scalar1=1.0, scalar2=1.0,
                    op0=ALU.min, op1=ALU.subtract,
                )
                nc.vector.tensor_tensor(out=zt[:], in0=r[:], in1=e[:], op=ALU.add)

            # ---------------- Stage 1: z1 = unfold(m1) @ f + kb ----------------
            l_sb = {}
            stats1 = [statp.tile([P, B_LOC, 6], F32, tag="st", name="st1") for _ in range(KC)]
            es_s1 = ExitStack()
            if True:
                fp = es_s1.enter_context(tc.tile_pool(name="f", bufs=W * KC))
                mp = es_s1.enter_context(tc.tile_pool(name="m1", bufs=B_LOC * KC))
                f_sb = {}
                for w in range(W):
                    for kc in range(KC):
                        t = fp.tile([P, E], F32, tag="f")
                        r0 = (w * KC + kc) * P
                        nc.sync.dma_start(out=t[:], in_=f_d[r0 : r0 + P, :])
                        f_sb[w, kc] = t
                m1_sb = {}
                for b in range(B_LOC):
                    for kc in range(KC):
                        t = mp.tile([P, L + 2 * S], F32, tag="m1")
                        nc.vector.memset(t[:, 0:S], 0.0)
                        nc.vector.memset(t[:, S + L : 2 * S + L], 0.0)
                        nc.sync.dma_start(
                            out=t[:, S : S + L], in_=m1t_d[b, kc * P : (kc + 1) * P, :]
                        )
                        m1_sb[b, kc] = t

                for b in range(B_LOC):
                    for mc in range(KC):
                        ps = psum.tile([P, L], F32, tag="ps")
                        n = 0
                        for w in range(W):
                            for kc in range(KC):
                                nc.tensor.matmul(
                                    ps[:],
                                    f_sb[w, kc][:, mc * P : (mc + 1) * P],
                                    m1_sb[b, kc][:, w : w + L],
                                    start=(n == 0),
                                    stop=(n == W * KC - 1),
                                )
                                n += 1
                        zt = lp.tile([P, L], F32, tag="l")
                        nc.vector.tensor_tensor(out=zt[:], in0=ps[:], in1=kbt_sb[mc][:], op=ALU.add)
                        nc.vector.bn_stats(out=stats1[mc][:, b, :], in_=zt[:])
                        l_sb[b, mc] = zt

            es_s1.close()

            pack1 = packp.tile([P, KC, 2], F32, tag="g")
            stats_to_pack(stats1, pack1)
            g1 = allreduce(pack1, KC * 2)
            sc1, bi1 = affines(g1, _G1, _B1)
            for b in range(B_LOC):
                for mc in range(KC):
                    elu_apply(l_sb[b, mc], sc1[mc], bi1[mc])

            # ------------- Stage 2/3: q2 = l@wq + qb, k2 = l@wk + kb -------------
            stats2 = [statp.tile([P, B_LOC, 6], F32, tag="st", name="st2") for _ in range(KC)]
            stats3 = [statp.tile([P, B_LOC, 6], F32, tag="st", name="st3") for _ in range(KC)]
            es_wqk = ExitStack()
            es_z = ExitStack()
            if True:
                z2p = es_z.enter_context(tc.tile_pool(name="z2", bufs=B_LOC * KC))
                z3p = es_z.enter_context(tc.tile_pool(name="z3", bufs=B_LOC * KC))
                wqkp = es_wqk.enter_context(tc.tile_pool(name="wqk", bufs=2 * KC))
                wq_sb, wk_sb = {}, {}
                for kc in range(KC):
                    t = wqkp.tile([P, E], F32, tag="wqk")
                    nc.sync.dma_start(out=t[:], in_=wq_d[kc * P : (kc + 1) * P, :])
                    wq_sb[kc] = t
                    t = wqkp.tile([P, E], F32, tag="wqk")
                    nc.sync.dma_start(out=t[:], in_=wk_d[kc * P : (kc + 1) * P, :])
                    wk_sb[kc] = t

                q2_sb, k2_sb = {}, {}
                for b in range(B_LOC):
                    for mc in range(KC):
                        ps = psum.tile([P, L], F32, tag="ps")
                        for kc in range(KC):
                            nc.tensor.matmul(
                                ps[:],
                                wq_sb[kc][:, mc * P : (mc + 1) * P],
                                l_sb[b, kc][:],
                                start=(kc == 0),
                                stop=(kc == KC - 1),
                            )
                        zt = z2p.tile([P, L], F32, tag="z2")
                        nc.vector.tensor_tensor(out=zt[:], in0=ps[:], in1=qbt_sb[mc][:], op=ALU.add)
                        nc.vector.bn_stats(out=stats2[mc][:, b, :], in_=zt[:])
                        q2_sb[b, mc] = zt

                        ps = psum.tile([P, L], F32, tag="ps")
                        for kc in range(KC):
                            nc.tensor.matmul(
                                ps[:],
                                wk_sb[kc][:, mc * P : (mc + 1) * P],
                                l_sb[b, kc][:],
                                start=(kc == 0),
                                stop=(kc == KC - 1),
                            )
                        zt = z3p.tile([P, L], F32, tag="z3")
                        nc.vector.tensor_tensor(out=zt[:], in0=ps[:], in1=kbt_sb[mc][:], op=ALU.add)
                        nc.vector.bn_stats(out=stats3[mc][:, b, :], in_=zt[:])
                        k2_sb[b, mc] = zt

                pack23 = packp.tile([P, 2 * KC, 2], F32, tag="g")
                for c, st in enumerate(stats2 + stats3):
                    mv = scr.tile([P, 2], F32, tag="scr")
                    nc.vector.bn_aggr(out=mv[:], in_=st[:])
                    sq = scr.tile([P, 1], F32, tag="scr")
                    nc.vector.tensor_mul(sq[:], mv[:, 0:1], mv[:, 0:1])
                    nc.vector.tensor_copy(pack23[:, c, 0:1], mv[:, 0:1])
                    nc.vector.tensor_add(pack23[:, c, 1:2], mv[:, 1:2], sq[:])
                nc.vector.tensor_scalar_mul(pack23[:], pack23[:], 1.0 / N_CORES)
                g23 = allreduce(pack23, 4 * KC)
                sc2, bi2 = affines(g23[:, 0 : 2 * KC], _G2, _B2)
                sc3, bi3 = affines(g23[:, 2 * KC : 4 * KC], _G3, _B3)

                for b in range(B_LOC):
                    for mc in range(KC):
                        elu_apply(q2_sb[b, mc], sc2[mc], bi2[mc])
                        elu_apply(k2_sb[b, mc], sc3[mc], bi3[mc])

                # ------------- Stage 4a: wT = (q2 @ k2^T)^T + wb^T -------------
                es_wqk.close()
                stats4 = [statp.tile([P, B_LOC, 6], F32, tag="st", name="st4") for _ in range(KC)]
                wt_sb = {}
                for b in range(B_LOC):
                    for kc in range(KC):
                        ps = psum.tile([P, L], F32, tag="ps")
                        for ec in range(KC):
                            nc.tensor.matmul(
                                ps[:],
                                k2_sb[b, ec][:, kc * P : (kc + 1) * P],
                                q2_sb[b, ec][:],
                                start=(ec == 0),
                                stop=(ec == KC - 1),
                            )
                        wt = wtp.tile([P, L], F32, tag="wT")
                        nc.vector.tensor_tensor(out=wt[:], in0=ps[:], in1=wbt_sb[kc][:], op=ALU.add)
                        nc.vector.bn_stats(out=stats4[kc][:, b, :], in_=wt[:])
                        wt_sb[b, kc] = wt

            es_z.close()

            pack4 = packp.tile([P, KC, 2], F32, tag="g")
            stats_to_pack(stats4, pack4)
            g4 = allreduce(pack4, KC * 2)
            sc4, bi4 = affines(g4, _G4, _B4)

            # ---------------- Stage 4b: BN4 + softmax over q ----------------
            for b in range(B_LOC):
                for kc in range(KC):
                    t = wt_sb[b, kc]
                    nc.vector.tensor_scalar(
                        out=t[:], in0=t[:], scalar1=sc4[kc][:], scalar2=bi4[kc][:],
                        op0=ALU.mult, op1=ALU.add,
                    )
                    mx = scr.tile([P, 1], F32, tag="scr")
                    nc.vector.tensor_reduce(out=mx[:], in_=t[:], axis=AX.X, op=ALU.max)
                    nm = scr.tile([P, 1], F32, tag="scr")
                    nc.vector.tensor_scalar_mul(nm[:], mx[:], -1.0)
                    ssum = scr.tile([P, 1], F32, tag="scr")
                    nc.scalar.activation(
                        out=t[:], in_=t[:], func=AF.Exp, bias=nm[:], accum_out=ssum[:]
                    )
                    rs = scr.tile([P, 1], F32, tag="scr")
                    nc.vector.reciprocal(rs[:], ssum[:])
                    nc.vector.tensor_scalar_mul(t[:], t[:], rs[:])

            # ---------------- Stage 5: out = w @ l, BN5 + ELU ----------------
            stats5 = [statp.tile([P, B_LOC, 6], F32, tag="st", name="st5") for _ in range(KC)]
            es_s5 = ExitStack()
            if True:
                lsp = es_s5.enter_context(tc.tile_pool(name="lstd", bufs=B_LOC * KC))
                outp = es_s5.enter_context(tc.tile_pool(name="out", bufs=B_LOC * KC))
                lstd_sb = {}
                for b in range(B_LOC):
                    for kc in range(KC):
                        lst = lsp.tile([P, E], F32, tag="lstd")
                        for mc in range(KC):
                            pst = psumT.tile([P, P], F32, tag="psT")
                            nc.tensor.transpose(
                                pst[:], l_sb[b, mc][:, kc * P : (kc + 1) * P], ident[:]
                            )
                            nc.vector.tensor_copy(lst[:, mc * P : (mc + 1) * P], pst[:])
                        lstd_sb[b, kc] = lst

                out_sb = {}
                for b in range(B_LOC):
                    for mc in range(KC):
                        ps = psum.tile([P, L], F32, tag="ps")
                        for kc in range(KC):
                            nc.tensor.matmul(
                                ps[:],
                                lstd_sb[b, kc][:, mc * P : (mc + 1) * P],
                                wt_sb[b, kc][:],
                                start=(kc == 0),
                                stop=(kc == KC - 1),
                            )
                        ot = outp.tile([P, L], F32, tag="out")
                        nc.vector.tensor_copy(ot[:], ps[:])
                        nc.vector.bn_stats(out=stats5[mc][:, b, :], in_=ot[:])
                        out_sb[b, mc] = ot

                pack5 = packp.tile([P, KC, 2], F32, tag="g")
                stats_to_pack(stats5, pack5)
                g5 = allreduce(pack5, KC * 2)
                sc5, bi5 = affines(g5, _G5, _B5)
                for b in range(B_LOC):
                    for mc in range(KC):
                        elu_apply(out_sb[b, mc], sc5[mc], bi5[mc])
                        nc.sync.dma_start(
                            out=out_d[b, mc * P : (mc + 1) * P, :], in_=out_sb[b, mc][:]
                        )

                es_s5.close()
                es_l.close()

    _split_waits(nc)
    _CACHE["nc"] = nc
    return nc


def _pack_affine(vecs):
    cols = []
    for v in vecs:
        cols.append(np.ascontiguousarray(np.asarray(v, np.float32).reshape(KC, P).T))
    return np.ascontiguousarray(np.concatenate(cols, axis=1))


def kernel(m1, f, wq, wk, qb, kb, wb, g1, b1, g2, b2, g3, b3, g4, b4, g5, b5):
    m1 = np.asarray(m1, np.float32)
    nc = _build()
    m1t = np.ascontiguousarray(m1.transpose(0, 2, 1))
    f_h = np.ascontiguousarray(np.asarray(f, np.float32))
    wq_h = np.ascontiguousarray(np.asarray(wq, np.float32))
    wk_h = np.ascontiguousarray(np.asarray(wk, np.float32))
    qbt = np.ascontiguousarray(np.asarray(qb, np.float32).T)
    kbt = np.ascontiguousarray(np.asarray(kb, np.float32).T)
    wbt = np.ascontiguousarray(np.asarray(wb, np.float32).T)
    gpack = _pack_affine([g1, b1, g2, b2, g3, b3, g4, b4, g5, b5])

    shared = {
        "f": f_h, "wq": wq_h, "wk": wk_h,
        "qbt": qbt, "kbt": kbt, "wbt": wbt, "gpack": gpack,
    }
    in_maps = [
        {"m1t": np.ascontiguousarray(m1t[i * B_LOC : (i + 1) * B_LOC]), **shared}
        for i in range(N_CORES)
    ]
    trace = os.environ.get("KERNEL_TRACE") == "1"
    res = run_bass_kernel_spmd(nc, in_maps, list(range(N_CORES)), trace=trace)
    _CACHE["last_results"] = res

    out = np.empty((B, L, E), np.float32)
    for i in range(N_CORES):
        out[i * B_LOC : (i + 1) * B_LOC] = res.results[i]["outt"].transpose(0, 2, 1)
    return out



# revision 14
# speedup vs baseline: 2.1137x; 2.1137x over previous
"""Bass/Tile TRN2 kernel for nn_LAN_4320737100678 (dense transformer block).

Data-parallel over the batch axis across 8 NeuronCores (4 batches/core).
Activations are feature-major ([E, L] per batch) so BatchNorm reductions
and the softmax run along the free axis. BN moments are globalized with
four in-kernel AllReduces (BN2+BN3 share a round) plus one warmup
AllReduce at t=0 that absorbs the CC-stream startup cost.

Perf structure vs the fp32 baseline:
 - stage-1 matmuls run in float32r (1 cyc/row vs 4 for fp32), stages 2-5
   and the l-transposes run in bf16.
 - stage-1 output is kept as l' = elu+1 (one Act pass + two vector
   passes); the -1 is folded into host-corrected stage-2/3 biases and
   into the transpose copy for stage 5.
 - rsqrt for the BN affines is Exp(-0.5*Ln(var+eps)) so every activation
   (Exp/Relu/Ln/Copy) lives in one act table -> no table reloads.
 - elementwise work is split across DVE / Pool / Act to keep each under
   the PE roofline.
"""

import os
import sys

sys.path.insert(0, "/opt/trn_rl_repo")

import ml_dtypes
import numpy as np

import concourse.bass as bass
import concourse.tile as tile
from concourse import mybir
from concourse.bass_utils import run_bass_kernel_spmd
from concourse.masks import make_identity

N_CORES = 8
B, L, E, W = 32, 512, 512, 5
S = W // 2
P = 128
KC = E // P            # feature chunks of 128
B_LOC = B // N_CORES   # batches per core
EPS = 1e-3
F32 = mybir.dt.float32
F32R = mybir.dt.float32r
BF16 = mybir.dt.bfloat16
AF = mybir.ActivationFunctionType
ALU = mybir.AluOpType
AX = mybir.AxisListType

# gpack column base offsets (each vector packed as [P, KC])
_G1, _B1, _G2, _B2, _G3, _B3, _G4, _B4, _G5, _B5 = (i * KC for i in range(10))

_MAX_CTRL_WAITS = 1


def _split_waits(nc, max_waits=_MAX_CTRL_WAITS):
    """walrus in this container encodes at most one sync-wait slot per
    instruction. Hoist extra waits onto same-engine NOPs inserted right
    before the owning instruction (same engine => executes first)."""
    for fn in nc.m.functions:
        for bb in fn.blocks:
            rebuilt = []
            changed = False
            for ins in bb.instructions:
                si = ins.sync_info
                if si is not None and len(si.on_wait) > max_waits:
                    waits = list(si.on_wait)
                    rest = waits[max_waits:]
                    for j in range(0, len(rest), max_waits):
                        nop = mybir.InstNoOp(
                            name=f"{ins.name}_wsplit{j}",
                            engine=ins.engine,
                            bass_nofuse=True,
                            sync_info=mybir.SyncInfo(
                                on_wait=rest[j : j + max_waits], on_update=[]
                            ),
                        )
                        rebuilt.append(nop)
                    ins.sync_info = mybir.SyncInfo(
                        on_wait=waits[:max_waits], on_update=list(si.on_update)
                    )
                    changed = True
                rebuilt.append(ins)
            if changed:
                bb.instructions = rebuilt


def _r(ap):
    return ap.bitcast(F32R)


_CACHE = {}


def _build():
    if "nc" in _CACHE:
        return _CACHE["nc"]
    from contextlib import ExitStack

    nc = bass.Bass("TRN2", target_bir_lowering=False, debug=False, num_devices=N_CORES)

    m1t_d = nc.dram_tensor("m1t", [B_LOC, E, L + 2 * S], F32R, kind="ExternalInput")
    f_d = nc.dram_tensor("f", [W * E, E], F32R, kind="ExternalInput")
    wq_d = nc.dram_tensor("wq", [E, E], BF16, kind="ExternalInput")
    wk_d = nc.dram_tensor("wk", [E, E], BF16, kind="ExternalInput")
    qbt_d = nc.dram_tensor("qbt", [E, L], F32, kind="ExternalInput")   # corrected
    kbt1_d = nc.dram_tensor("kbt1", [E, L], F32, kind="ExternalInput")  # exact kb^T
    kbt3_d = nc.dram_tensor("kbt3", [E, L], F32, kind="ExternalInput")  # corrected
    wbt_d = nc.dram_tensor("wbt", [L, L], F32, kind="ExternalInput")
    gp_d = nc.dram_tensor("gpack", [P, 10 * KC], F32, kind="ExternalInput")
    out_d = nc.dram_tensor("outt", [B_LOC, E, L], F32, kind="ExternalOutput")

    groups = [list(range(N_CORES))]

    with tile.TileContext(nc) as tc:
        with (
            tc.tile_pool(name="const", bufs=1) as const,
            tc.tile_pool(name="aff", bufs=16) as affp,
            tc.tile_pool(name="stats", bufs=24) as statp,
            tc.tile_pool(name="packs", bufs=8) as packp,
            tc.tile_pool(name="scr", bufs=24) as scr,
            tc.tile_pool(name="mx", bufs=B_LOC * KC) as mxp,
            tc.tile_pool(name="ebuf", bufs=6) as ebuf,
            tc.tile_pool(name="l", bufs=B_LOC * KC) as lp,
            tc.tile_pool(name="wT", bufs=B_LOC * KC) as wtp,
            tc.tile_pool(name="lstd", bufs=B_LOC * KC) as lsp,
            tc.tile_pool(name="wqk", bufs=2 * KC) as wqkp,
            tc.tile_pool(name="wbt", bufs=KC) as wbtp,
            tc.tile_pool(name="qkb", bufs=2 * KC) as qk_biasp,
            tc.tile_pool(name="dram", bufs=12, space="DRAM") as dram,
        ):
            # ---- warmup AllReduce: absorbs CC-stream startup + syncs cores
            warm = const.tile([P, 2], F32, tag="warm")
            nc.vector.memset(warm[:], 0.0)
            cc_w_in = dram.tile([P, 2], F32, tag="cc")
            cc_w_out = dram.tile([P, 2], F32, tag="cc")
            nc.sync.dma_start(out=cc_w_in[:], in_=warm[:])
            nc.gpsimd.collective_compute(
                "AllReduce", ALU.add, replica_groups=groups,
                ins=[cc_w_in.opt()], outs=[cc_w_out.opt()],
            )

            # ---- constants
            ident = const.tile([P, P], BF16, tag="ident")
            make_identity(nc, ident[:])
            epst = const.tile([P, 1], F32, tag="eps")
            nc.vector.memset(epst[:], EPS)

            # ---- stage-1-scoped pools (LIFO: closed before stage-2 pools open)
            es_B = ExitStack()
            z1p = es_B.enter_context(tc.tile_pool(name="z1", bufs=B_LOC * KC))
            kb1p = es_B.enter_context(tc.tile_pool(name="kb1", bufs=KC))
            fp = es_B.enter_context(tc.tile_pool(name="f", bufs=W * KC))
            mp = es_B.enter_context(tc.tile_pool(name="m1", bufs=B_LOC * KC))

            # stage-1 inputs, interleaved by kc so PE can start early
            f_sb, m1_sb = {}, {}
            for kc in range(KC):
                for b in range(B_LOC):
                    t = mp.tile([P, L + 2 * S], F32R, tag="m1")
                    nc.sync.dma_start(
                        out=t[:], in_=m1t_d[b, kc * P : (kc + 1) * P, :]
                    )
                    m1_sb[b, kc] = t
                for w in range(W):
                    t = fp.tile([P, E], F32R, tag="f")
                    r0 = (w * KC + kc) * P
                    nc.sync.dma_start(out=t[:], in_=f_d[r0 : r0 + P, :])
                    f_sb[w, kc] = t

            # weights / biases that can trickle in during stage 1 (Act queue)
            wq_sb, wk_sb, kbt1_sb, wbt_sb, qbt_sb, kbt3_sb = {}, {}, {}, {}, {}, {}
            for kc in range(KC):
                t = wqkp.tile([P, E], BF16, tag="wqk")
                nc.scalar.dma_start(out=t[:], in_=wq_d[kc * P : (kc + 1) * P, :])
                wq_sb[kc] = t
                t = wqkp.tile([P, E], BF16, tag="wqk")
                nc.scalar.dma_start(out=t[:], in_=wk_d[kc * P : (kc + 1) * P, :])
                wk_sb[kc] = t
                t = kb1p.tile([P, L], F32, tag="kb1")
                nc.scalar.dma_start(out=t[:], in_=kbt1_d[kc * P : (kc + 1) * P, :])
                kbt1_sb[kc] = t
            for kc in range(KC):
                t = wbtp.tile([P, L], F32, tag="wbt")
                nc.scalar.dma_start(out=t[:], in_=wbt_d[kc * P : (kc + 1) * P, :])
                wbt_sb[kc] = t
                t = qk_biasp.tile([P, L], F32, tag="qkb")
                nc.scalar.dma_start(out=t[:], in_=qbt_d[kc * P : (kc + 1) * P, :])
                qbt_sb[kc] = t
                t = qk_biasp.tile([P, L], F32, tag="qkb")
                nc.scalar.dma_start(out=t[:], in_=kbt3_d[kc * P : (kc + 1) * P, :])
                kbt3_sb[kc] = t
            gp = const.tile([P, 10 * KC], F32, tag="gp")
            nc.scalar.dma_start(out=gp[:], in_=gp_d[:])

            # ---------------- helpers ----------------
            def allreduce(pack, width):
                cc_in = dram.tile([P, width], F32, tag="cc")
                cc_out = dram.tile([P, width], F32, tag="cc")
                nc.sync.dma_start(out=cc_in[:], in_=pack[:])
                nc.gpsimd.collective_compute(
                    "AllReduce", ALU.add, replica_groups=groups,
                    ins=[cc_in.opt()], outs=[cc_out.opt()],
                )
                g = packp.tile([P, width], F32, tag="g")
                nc.sync.dma_start(out=g[:], in_=cc_out[:])
                return g

            def stats_to_pack(stats_tiles, pack):
                """stats_tiles: per-chunk [P, B_LOC, 6] bn_stats rows.
                pack[:, c, 0] = local_mean/8, pack[:, c, 1] = local_E[x^2]/8."""
                for c, st in enumerate(stats_tiles):
                    mv = scr.tile([P, 2], F32, tag="scr")
                    nc.vector.bn_aggr(out=mv[:], in_=st[:])
                    sq = scr.tile([P, 1], F32, tag="scr")
                    nc.vector.tensor_mul(sq[:], mv[:, 0:1], mv[:, 0:1])
                    nc.vector.tensor_copy(pack[:, c, 0:1], mv[:, 0:1])
                    nc.vector.tensor_add(pack[:, c, 1:2], mv[:, 1:2], sq[:])
                nc.vector.tensor_scalar_mul(pack[:], pack[:], 1.0 / N_CORES)

            def affines(g, nch, gcol, bcol, plus_one=False):
                """From allreduced [P, nch, 2] (mean, E[x^2]) compute [P, nch]
                scale = gamma*rsqrt(var+eps), bias = beta - mean*scale.
                rsqrt = Exp(-0.5*Ln(var+eps)) (stays in the exp act table)."""
                gv = g.rearrange("p (c two) -> p c two", two=2)
                mean = gv[:, :, 0]
                ex2 = gv[:, :, 1]
                msq = scr.tile([P, nch], F32, tag="scr")
                nc.vector.tensor_mul(msq[:], mean, mean)
                var = scr.tile([P, nch], F32, tag="scr")
                nc.vector.tensor_sub(var[:], ex2, msq[:])
                lnv = scr.tile([P, nch], F32, tag="scr")
                nc.scalar.activation(out=lnv[:], in_=var[:], func=AF.Ln, bias=epst[:])
                rinv = scr.tile([P, nch], F32, tag="scr")
                nc.scalar.activation(out=rinv[:], in_=lnv[:], func=AF.Exp, scale=-0.5)
                sc = affp.tile([P, nch], F32, tag="aff")
                nc.vector.tensor_mul(sc[:], rinv[:], gp[:, gcol : gcol + nch])
                tb = scr.tile([P, nch], F32, tag="scr")
                nc.vector.tensor_mul(tb[:], mean, sc[:])
                bi = affp.tile([P, nch], F32, tag="aff")
                nc.vector.tensor_sub(bi[:], gp[:, bcol : bcol + nch], tb[:])
                if not plus_one:
                    return sc, bi, None
                bip = affp.tile([P, nch], F32, tag="aff")
                nc.vector.tensor_scalar_add(bip[:], bi[:], 1.0)
                return sc, bi, bip

            # ---------------- Stage 1: z1 = unfold(m1) @ f + kb ----------------
            # fp32r matmuls, two 8-bank PSUM waves, contraction-outer order so
            # PE consumes f tiles in DMA arrival order.
            z1_sb = {}
            stats1 = [statp.tile([P, B_LOC, 6], F32, tag="st", name="st1") for _ in range(KC)]
            es_ps1 = ExitStack()
            ps1 = es_ps1.enter_context(tc.tile_pool(name="ps1", bufs=8, space="PSUM"))
            for wave in (0, 1):
                mcs = (2 * wave, 2 * wave + 1)
                ps = {}
                for b in range(B_LOC):
                    for mc in mcs:
                        ps[b, mc] = ps1.tile([P, L], F32, tag="ps", name=f"ps1_{b}_{mc}")
                for kc in range(KC):
                    for w in range(W):
                        first = kc == 0 and w == 0
                        last = kc == KC - 1 and w == W - 1
                        for b in range(B_LOC):
                            for mc in mcs:
                                nc.tensor.matmul(
                                    ps[b, mc][:],
                                    f_sb[w, kc][:, mc * P : (mc + 1) * P],
                                    m1_sb[b, kc][:, w : w + L],
                                    start=first,
                                    stop=last,
                                )
                for b in range(B_LOC):
                    for mc in mcs:
                        zt = z1p.tile([P, L], BF16, tag="z1")
                        nc.vector.tensor_tensor(
                            out=zt[:], in0=ps[b, mc][:], in1=kbt1_sb[mc][:], op=ALU.add
                        )
                        nc.vector.bn_stats(out=stats1[mc][:, b, :], in_=zt[:])
                        z1_sb[b, mc] = zt

            pack1 = packp.tile([P, KC, 2], F32, tag="g")
            stats_to_pack(stats1, pack1)
            g1 = allreduce(pack1, KC * 2)
            sc1, bi1, bip1 = affines(g1, KC, _G1, _B1, plus_one=True)

            # elu': l' = elu(y)+1 = max(y+1, min(exp(y), 1)), y = sc*z + bi
            l_sb = {}
            for b in range(B_LOC):
                for mc in range(KC):
                    z = z1_sb[b, mc]
                    e = ebuf.tile([P, L], BF16, tag="e")
                    nc.scalar.activation(
                        out=e[:], in_=z[:], func=AF.Exp,
                        bias=bi1[:, mc : mc + 1], scale=sc1[:, mc : mc + 1],
                    )
                    y1 = ebuf.tile([P, L], BF16, tag="e")
                    nc.vector.tensor_scalar(
                        out=y1[:], in0=z[:],
                        scalar1=sc1[:, mc : mc + 1], scalar2=bip1[:, mc : mc + 1],
                        op0=ALU.mult, op1=ALU.add,
                    )
                    lt = lp.tile([P, L], BF16, tag="l")
                    nc.vector.scalar_tensor_tensor(
                        out=lt[:], in0=e[:], scalar=1.0, in1=y1[:],
                        op0=ALU.min, op1=ALU.max,
                    )
                    l_sb[b, mc] = lt

            es_ps1.close()
            es_B.close()

            # ---- stage-2..4-scoped pools
            es_C = ExitStack()
            z23p = es_C.enter_context(tc.tile_pool(name="z23", bufs=2 * B_LOC * KC))
            qkp = es_C.enter_context(tc.tile_pool(name="qk", bufs=2 * B_LOC * KC))
            es_psB = ExitStack()
            psB = es_psB.enter_context(tc.tile_pool(name="psB", bufs=4, space="PSUM"))
            psT = es_psB.enter_context(tc.tile_pool(name="psT", bufs=2, space="PSUM"))

            # ------------- Stage 2/3: q2 = l@wq + qb', k2 = l@wk + kb' -------------
            stats2 = [statp.tile([P, B_LOC, 6], F32, tag="st", name="st2") for _ in range(KC)]
            stats3 = [statp.tile([P, B_LOC, 6], F32, tag="st", name="st3") for _ in range(KC)]
            z2_sb, z3_sb = {}, {}
            for b in range(B_LOC):
                for mc in range(KC):
                    for w_sb, bias_sb, zdst, stats, move_eng in (
                        (wq_sb, qbt_sb, z2_sb, stats2, nc.vector),
                        (wk_sb, kbt3_sb, z3_sb, stats3, nc.vector),
                    ):
                        ps = psB.tile([P, L], F32, tag="ps")
                        for kc in range(KC):
                            nc.tensor.matmul(
                                ps[:],
                                w_sb[kc][:, mc * P : (mc + 1) * P],
                                l_sb[b, kc][:],
                                start=(kc == 0),
                                stop=(kc == KC - 1),
                            )
                        zt = z23p.tile([P, L], BF16, tag="z23")
                        move_eng.tensor_tensor(
                            out=zt[:], in0=ps[:], in1=bias_sb[mc][:], op=ALU.add
                        )
                        nc.vector.bn_stats(out=stats[mc][:, b, :], in_=zt[:])
                        zdst[b, mc] = zt

            pack23 = packp.tile([P, 2 * KC, 2], F32, tag="g")
            stats_to_pack(stats2 + stats3, pack23)
            g23 = allreduce(pack23, 4 * KC)
            sc2, bi2, _ = affines(g23[:, 0 : 2 * KC], KC, _G2, _B2)
            sc3, bi3, _ = affines(g23[:, 2 * KC : 4 * KC], KC, _G3, _B3)

            # transposes of l' for stage 5 fill the AR2/3 PE-idle window
            lstd_sb = {}
            for b in range(B_LOC):
                for kc in range(KC):
                    pst = psT.tile([P, L], BF16, tag="psT")
                    for mc in range(KC):
                        nc.tensor.transpose(
                            pst[:, mc * P : (mc + 1) * P],
                            l_sb[b, mc][:, kc * P : (kc + 1) * P],
                            ident[:],
                        )
                    lst = lsp.tile([P, E], BF16, tag="lstd")
                    nc.vector.tensor_scalar_add(lst[:], pst[:], -1.0)
                    lstd_sb[b, kc] = lst

            # elu for q2/k2 (exact): e=Exp(y), r=Relu(y), out = r + (min(e,1)-1)
            q2_sb, k2_sb = {}, {}
            for b in range(B_LOC):
                for mc in range(KC):
                    for z, sc, bi, dst in (
                        (z2_sb[b, mc], sc2, bi2, q2_sb),
                        (z3_sb[b, mc], sc3, bi3, k2_sb),
                    ):
                        e = ebuf.tile([P, L], BF16, tag="e")
                        nc.scalar.activation(
                            out=e[:], in_=z[:], func=AF.Exp,
                            bias=bi[:, mc : mc + 1], scale=sc[:, mc : mc + 1],
                        )
                        r = ebuf.tile([P, L], BF16, tag="e")
                        nc.scalar.activation(
                            out=r[:], in_=z[:], func=AF.Relu,
                            bias=bi[:, mc : mc + 1], scale=sc[:, mc : mc + 1],
                        )
                        t = ebuf.tile([P, L], BF16, tag="e")
                        nc.vector.tensor_scalar(
                            out=t[:], in0=e[:], scalar1=1.0, scalar2=1.0,
                            op0=ALU.min, op1=ALU.subtract,
                        )
                        o = qkp.tile([P, L], BF16, tag="qk")
                        nc.gpsimd.tensor_tensor(out=o[:], in0=r[:], in1=t[:], op=ALU.add)
                        dst[b, mc] = o

            # ------------- Stage 4: wT = (q2 @ k2^T)^T + wb^T, BN4, softmax -------------
            stats4 = [statp.tile([P, B_LOC, 6], F32, tag="st", name="st4") for _ in range(KC)]
            wt_sb, mx_sb = {}, {}
            for b in range(B_LOC):
                for kc in range(KC):
                    ps = psB.tile([P, L], F32, tag="ps")
                    for ec in range(KC):
                        nc.tensor.matmul(
                            ps[:],
                            k2_sb[b, ec][:, kc * P : (kc + 1) * P],
                            q2_sb[b, ec][:],
                            start=(ec == 0),
                            stop=(ec == KC - 1),
                        )
                    wt = wtp.tile([P, L], BF16, tag="wT")
                    nc.vector.tensor_tensor(out=wt[:], in0=ps[:], in1=wbt_sb[kc][:], op=ALU.add)
                    nc.vector.bn_stats(out=stats4[kc][:, b, :], in_=wt[:])
                    mx = mxp.tile([P, 1], F32, tag="mx")
                    nc.vector.tensor_reduce(out=mx[:], in_=wt[:], axis=AX.X, op=ALU.max)
                    wt_sb[b, kc] = wt
                    mx_sb[b, kc] = mx

            pack4 = packp.tile([P, KC, 2], F32, tag="g")
            stats_to_pack(stats4, pack4)
            g4 = allreduce(pack4, KC * 2)
            sc4, bi4, _ = affines(g4, KC, _G4, _B4)

            # softmax over q (free axis): exp(sc4*x - sc4*mx), scale by 1/sum
            for b in range(B_LOC):
                for kc in range(KC):
                    wt = wt_sb[b, kc]
                    eb = scr.tile([P, 1], F32, tag="scr")
                    nc.vector.tensor_scalar(
                        out=eb[:], in0=mx_sb[b, kc][:],
                        scalar1=sc4[:, kc : kc + 1], scalar2=-1.0,
                        op0=ALU.mult, op1=ALU.mult,
                    )
                    ssum = scr.tile([P, 1], F32, tag="scr")
                    nc.scalar.activation(
                        out=wt[:], in_=wt[:], func=AF.Exp,
                        bias=eb[:], scale=sc4[:, kc : kc + 1], accum_out=ssum[:],
                    )
                    rs = scr.tile([P, 1], F32, tag="scr")
                    nc.vector.reciprocal(rs[:], ssum[:])
                    nc.vector.tensor_scalar_mul(wt[:], wt[:], rs[:])

            es_C.close()

            # ---------------- Stage 5: out = w @ l, BN5 + ELU ----------------
            es_D = ExitStack()
            z5p = es_D.enter_context(tc.tile_pool(name="z5", bufs=B_LOC * KC))
            outp = es_D.enter_context(tc.tile_pool(name="out", bufs=B_LOC * KC))
            stats5 = [statp.tile([P, B_LOC, 6], F32, tag="st", name="st5") for _ in range(KC)]
            z5_sb = {}
            for b in range(B_LOC):
                for mc in range(KC):
                    ps = psB.tile([P, L], F32, tag="ps")
                    for kc in range(KC):
                        nc.tensor.matmul(
                            ps[:],
                            lstd_sb[b, kc][:, mc * P : (mc + 1) * P],
                            wt_sb[b, kc][:],
                            start=(kc == 0),
                            stop=(kc == KC - 1),
                        )
                    zt = z5p.tile([P, L], BF16, tag="z5")
                    nc.scalar.activation(out=zt[:], in_=ps[:], func=AF.Copy)
                    nc.vector.bn_stats(out=stats5[mc][:, b, :], in_=zt[:])
                    z5_sb[b, mc] = zt

            pack5 = packp.tile([P, KC, 2], F32, tag="g")
            stats_to_pack(stats5, pack5)
            g5 = allreduce(pack5, KC * 2)
            sc5, bi5, _ = affines(g5, KC, _G5, _B5)

            for b in range(B_LOC):
                for mc in range(KC):
                    z = z5_sb[b, mc]
                    e = ebuf.tile([P, L], BF16, tag="e")
                    nc.scalar.activation(
                        out=e[:], in_=z[:], func=AF.Exp,
                        bias=bi5[:, mc : mc + 1], scale=sc5[:, mc : mc + 1],
                    )
                    r = ebuf.tile([P, L], BF16, tag="e")
                    nc.scalar.activation(
                        out=r[:], in_=z[:], func=AF.Relu,
                        bias=bi5[:, mc : mc + 1], scale=sc5[:, mc : mc + 1],
                    )
                    t = ebuf.tile([P, L], BF16, tag="e")
                    nc.vector.tensor_scalar(
                        out=t[:], in0=e[:], scalar1=1.0, scalar2=1.0,
                        op0=ALU.min, op1=ALU.subtract,
                    )
                    ot = outp.tile([P, L], F32, tag="out")
                    nc.gpsimd.tensor_tensor(out=ot[:], in0=r[:], in1=t[:], op=ALU.add)
                    nc.sync.dma_start(
                        out=out_d[b, mc * P : (mc + 1) * P, :], in_=ot[:]
                    )

            es_D.close()
            es_psB.close()

    _split_waits(nc)
    _CACHE["nc"] = nc
    return nc


def _pack_affine(vecs):
    cols = []
    for v in vecs:
        cols.append(np.ascontiguousarray(np.asarray(v, np.float32).reshape(KC, P).T))
    return np.ascontiguousarray(np.concatenate(cols, axis=1))


def kernel(m1, f, wq, wk, qb, kb, wb, g1, b1, g2, b2, g3, b3, g4, b4, g5, b5):
    m1 = np.asarray(m1, np.float32)
    nc = _build()
    m1t_ = m1.transpose(0, 2, 1)
    m1t = np.zeros((B, E, L + 2 * S), np.float32)
    m1t[:, :, S : S + L] = m1t_
    f_h = np.ascontiguousarray(np.asarray(f, np.float32))
    wq_bf = np.asarray(wq, np.float32).astype(ml_dtypes.bfloat16)
    wk_bf = np.asarray(wk, np.float32).astype(ml_dtypes.bfloat16)
    # stage-1 output is l' = elu+1; fold the -1 into the stage-2/3 biases
    # using the column sums of the bf16-rounded weights (what the HW adds).
    qb_c = np.asarray(qb, np.float32) - wq_bf.astype(np.float32).sum(axis=0)[None, :]
    kb_c = np.asarray(kb, np.float32) - wk_bf.astype(np.float32).sum(axis=0)[None, :]
    qbt = np.ascontiguousarray(qb_c.T)
    kbt1 = np.ascontiguousarray(np.asarray(kb, np.float32).T)
    kbt3 = np.ascontiguousarray(kb_c.T)
    wbt = np.ascontiguousarray(np.asarray(wb, np.float32).T)
    gpack = _pack_affine([g1, b1, g2, b2, g3, b3, g4, b4, g5, b5])

    shared = {
        "f": f_h, "wq": wq_bf, "wk": wk_bf,
        "qbt": qbt, "kbt1": kbt1, "kbt3": kbt3, "wbt": wbt, "gpack": gpack,
    }
    in_maps = [
        {"m1t": np.ascontiguousarray(m1t[i * B_LOC : (i + 1) * B_LOC]), **shared}
        for i in range(N_CORES)
    ]
    trace = os.environ.get("KERNEL_TRACE") == "1"
    res = run_bass_kernel_spmd(nc, in_maps, list(range(N_CORES)), trace=trace)
    _CACHE["last_results"] = res

    out = np.empty((B, L, E), np.float32)
    for i in range(N_CORES):
        out[i * B_LOC : (i + 1) * B_LOC] = res.results[i]["outt"].transpose(0, 2, 1)
    return out


# revision 17
# speedup vs baseline: 2.2215x; 1.0510x over previous
"""Bass/Tile TRN2 kernel for nn_LAN_4320737100678 (dense transformer block).

Data-parallel over the batch axis across 8 NeuronCores (4 batches/core).
Activations are feature-major ([E, L] per batch) so BatchNorm reductions
and the softmax run along the free axis. BN moments are globalized with
four in-kernel AllReduces (BN2+BN3 share a round) plus one warmup
AllReduce at t=0 that absorbs the CC-stream startup cost.

Perf structure vs the fp32 baseline:
 - stage-1 matmuls run in float32r (1 cyc/row vs 4 for fp32), stages 2-5
   and the l-transposes run in bf16.
 - stage-1 output is kept as l' = elu+1 (one Act pass + two vector
   passes); the -1 is folded into host-corrected stage-2/3 biases and
   into the transpose copy for stage 5.
 - rsqrt for the BN affines is Exp(-0.5*Ln(var+eps)) so every activation
   (Exp/Relu/Ln/Copy) lives in one act table -> no table reloads.
 - elementwise work is split across DVE / Pool / Act to keep each under
   the PE roofline.
"""

import os
import sys

sys.path.insert(0, "/opt/trn_rl_repo")

import ml_dtypes
import numpy as np

import concourse.bass as bass
import concourse.tile as tile
from concourse import mybir
from concourse.bass_utils import run_bass_kernel_spmd
from concourse.masks import make_identity

N_CORES = 8
B, L, E, W = 32, 512, 512, 5
S = W // 2
P = 128
KC = E // P            # feature chunks of 128
B_LOC = B // N_CORES   # batches per core
EPS = 1e-3
F32 = mybir.dt.float32
F32R = mybir.dt.float32r
BF16 = mybir.dt.bfloat16
AF = mybir.ActivationFunctionType
ALU = mybir.AluOpType
AX = mybir.AxisListType

# gpack column base offsets (each vector packed as [P, KC])
_G1, _B1, _G2, _B2, _G3, _B3, _G4, _B4, _G5, _B5 = (i * KC for i in range(10))

_MAX_CTRL_WAITS = 1


def _split_waits(nc, max_waits=_MAX_CTRL_WAITS):
    """walrus in this container encodes at most one sync-wait slot per
    instruction. Hoist extra waits onto same-engine NOPs inserted right
    before the owning instruction (same engine => executes first)."""
    for fn in nc.m.functions:
        for bb in fn.blocks:
            rebuilt = []
            changed = False
            for ins in bb.instructions:
                si = ins.sync_info
                if si is not None and len(si.on_wait) > max_waits:
                    waits = list(si.on_wait)
                    rest = waits[max_waits:]
                    for j in range(0, len(rest), max_waits):
                        nop = mybir.InstNoOp(
                            name=f"{ins.name}_wsplit{j}",
                            engine=ins.engine,
                            bass_nofuse=True,
                            sync_info=mybir.SyncInfo(
                                on_wait=rest[j : j + max_waits], on_update=[]
                            ),
                        )
                        rebuilt.append(nop)
                    ins.sync_info = mybir.SyncInfo(
                        on_wait=waits[:max_waits], on_update=list(si.on_update)
                    )
                    changed = True
                rebuilt.append(ins)
            if changed:
                bb.instructions = rebuilt


def _r(ap):
    return ap.bitcast(F32R)


_CACHE = {}


def _build():
    if "nc" in _CACHE:
        return _CACHE["nc"]
    from contextlib import ExitStack

    nc = bass.Bass("TRN2", target_bir_lowering=False, debug=False, num_devices=N_CORES)

    m1t_d = nc.dram_tensor("m1t", [B_LOC, E, L + 2 * S], F32R, kind="ExternalInput")
    f_d = nc.dram_tensor("f", [W * E, E], F32R, kind="ExternalInput")
    wq_d = nc.dram_tensor("wq", [E, E], BF16, kind="ExternalInput")
    wk_d = nc.dram_tensor("wk", [E, E], BF16, kind="ExternalInput")
    qbt_d = nc.dram_tensor("qbt", [E, L], F32, kind="ExternalInput")   # corrected
    kbt1_d = nc.dram_tensor("kbt1", [E, L], F32, kind="ExternalInput")  # exact kb^T
    kbt3_d = nc.dram_tensor("kbt3", [E, L], F32, kind="ExternalInput")  # corrected
    wbt_d = nc.dram_tensor("wbt", [L, L], F32, kind="ExternalInput")
    gp_d = nc.dram_tensor("gpack", [P, 10 * KC], F32, kind="ExternalInput")
    out_d = nc.dram_tensor("outt", [B_LOC, E, L], F32, kind="ExternalOutput")

    groups = [list(range(N_CORES))]

    with tile.TileContext(nc) as tc:
        with (
            tc.tile_pool(name="const", bufs=1) as const,
            tc.tile_pool(name="aff", bufs=16) as affp,
            tc.tile_pool(name="stats", bufs=24) as statp,
            tc.tile_pool(name="packs", bufs=8) as packp,
            tc.tile_pool(name="scr", bufs=24) as scr,
            tc.tile_pool(name="mx", bufs=B_LOC * KC) as mxp,
            tc.tile_pool(name="ebuf", bufs=6) as ebuf,
            tc.tile_pool(name="l", bufs=B_LOC * KC) as lp,
            tc.tile_pool(name="wT", bufs=B_LOC * KC) as wtp,
            tc.tile_pool(name="lstd", bufs=B_LOC * KC) as lsp,
            tc.tile_pool(name="wqk", bufs=2 * KC) as wqkp,
            tc.tile_pool(name="wbt", bufs=KC) as wbtp,
            tc.tile_pool(name="qkb", bufs=2 * KC) as qk_biasp,
            tc.tile_pool(name="dram", bufs=12, space="DRAM") as dram,
        ):
            # ---- warmup AllReduce: absorbs CC-stream startup + syncs cores
            warm = const.tile([P, 2], F32, tag="warm")
            nc.vector.memset(warm[:], 0.0)
            cc_w_in = dram.tile([P, 2], F32, tag="cc")
            cc_w_out = dram.tile([P, 2], F32, tag="cc")
            nc.sync.dma_start(out=cc_w_in[:], in_=warm[:])
            nc.gpsimd.collective_compute(
                "AllReduce", ALU.add, replica_groups=groups,
                ins=[cc_w_in.opt()], outs=[cc_w_out.opt()],
            )

            # ---- constants
            ident = const.tile([P, P], BF16, tag="ident")
            make_identity(nc, ident[:])
            epst = const.tile([P, 1], F32, tag="eps")
            nc.vector.memset(epst[:], EPS)

            # ---- stage-1-scoped pools (LIFO: closed before stage-2 pools open)
            es_B = ExitStack()
            z1p = es_B.enter_context(tc.tile_pool(name="z1", bufs=B_LOC * KC))
            kb1p = es_B.enter_context(tc.tile_pool(name="kb1", bufs=KC))
            fp = es_B.enter_context(tc.tile_pool(name="f", bufs=W * KC))
            mp = es_B.enter_context(tc.tile_pool(name="m1", bufs=B_LOC * KC))

            # stage-1 inputs, interleaved by kc so PE can start early
            f_sb, m1_sb = {}, {}

            def load_f(w, kc):
                t = fp.tile([P, E], F32R, tag="f", name=f"f_{w}_{kc}")
                r0 = (w * KC + kc) * P
                nc.sync.dma_start(out=t[:], in_=f_d[r0 : r0 + P, :])
                f_sb[w, kc] = t

            def load_m1(b, kc):
                t = mp.tile([P, L + 2 * S], F32R, tag="m1", name=f"m1_{b}_{kc}")
                nc.sync.dma_start(out=t[:], in_=m1t_d[b, kc * P : (kc + 1) * P, :])
                m1_sb[b, kc] = t

            for kc in range(KC):
                load_f(0, kc)
                load_m1(0, kc)
                for b in range(1, B_LOC):
                    load_m1(b, kc)
                for w in range(1, W):
                    load_f(w, kc)

            # weights / biases that can trickle in during stage 1 (Act queue)
            wq_sb, wk_sb, kbt1_sb, wbt_sb, qbt_sb, kbt3_sb = {}, {}, {}, {}, {}, {}
            for kc in range(KC):
                t = wqkp.tile([P, E], BF16, tag="wqk")
                nc.scalar.dma_start(out=t[:], in_=wq_d[kc * P : (kc + 1) * P, :])
                wq_sb[kc] = t
                t = wqkp.tile([P, E], BF16, tag="wqk")
                nc.scalar.dma_start(out=t[:], in_=wk_d[kc * P : (kc + 1) * P, :])
                wk_sb[kc] = t
                t = kb1p.tile([P, L], F32, tag="kb1")
                nc.scalar.dma_start(out=t[:], in_=kbt1_d[kc * P : (kc + 1) * P, :])
                kbt1_sb[kc] = t
            for kc in range(KC):
                t = wbtp.tile([P, L], F32, tag="wbt")
                nc.scalar.dma_start(out=t[:], in_=wbt_d[kc * P : (kc + 1) * P, :])
                wbt_sb[kc] = t
                t = qk_biasp.tile([P, L], F32, tag="qkb")
                nc.scalar.dma_start(out=t[:], in_=qbt_d[kc * P : (kc + 1) * P, :])
                qbt_sb[kc] = t
                t = qk_biasp.tile([P, L], F32, tag="qkb")
                nc.scalar.dma_start(out=t[:], in_=kbt3_d[kc * P : (kc + 1) * P, :])
                kbt3_sb[kc] = t
            gp = const.tile([P, 10 * KC], F32, tag="gp")
            nc.scalar.dma_start(out=gp[:], in_=gp_d[:])

            # ---------------- helpers ----------------
            def allreduce(pack, width):
                cc_in = dram.tile([P, width], F32, tag="cc")
                cc_out = dram.tile([P, width], F32, tag="cc")
                nc.sync.dma_start(out=cc_in[:], in_=pack[:])
                nc.gpsimd.collective_compute(
                    "AllReduce", ALU.add, replica_groups=groups,
                    ins=[cc_in.opt()], outs=[cc_out.opt()],
                )
                g = packp.tile([P, width], F32, tag="g")
                nc.sync.dma_start(out=g[:], in_=cc_out[:])
                return g

            def stats_to_pack(stats_tiles, pack):
                """stats_tiles: per-chunk [P, B_LOC, 6] bn_stats rows.
                pack[:, c, 0] = local_mean/8, pack[:, c, 1] = local_E[x^2]/8."""
                n = len(stats_tiles)
                for c, st in enumerate(stats_tiles):
                    nc.vector.bn_aggr(out=pack[:, c, :], in_=st[:])
                mean = pack[:, :, 0]
                ex2 = pack[:, :, 1]
                msq = scr.tile([P, n], F32, tag="scr")
                nc.vector.tensor_mul(msq[:], mean, mean)
                nc.vector.tensor_add(ex2, ex2, msq[:])
                nc.vector.tensor_scalar_mul(pack[:], pack[:], 1.0 / N_CORES)

            def affines(g, nch, gcol, bcol, plus_one=False):
                """From allreduced [P, nch, 2] (mean, E[x^2]) compute [P, nch]
                scale = gamma*rsqrt(var+eps), bias = beta - mean*scale.
                rsqrt = Exp(-0.5*Ln(var+eps)) (stays in the exp act table)."""
                gv = g.rearrange("p (c two) -> p c two", two=2)
                mean = gv[:, :, 0]
                ex2 = gv[:, :, 1]
                msq = scr.tile([P, nch], F32, tag="scr")
                nc.vector.tensor_mul(msq[:], mean, mean)
                var = scr.tile([P, nch], F32, tag="scr")
                nc.vector.tensor_sub(var[:], ex2, msq[:])
                lnv = scr.tile([P, nch], F32, tag="scr")
                nc.scalar.activation(out=lnv[:], in_=var[:], func=AF.Ln, bias=epst[:])
                rinv = scr.tile([P, nch], F32, tag="scr")
                nc.scalar.activation(out=rinv[:], in_=lnv[:], func=AF.Exp, scale=-0.5)
                sc = affp.tile([P, nch], F32, tag="aff")
                nc.vector.tensor_mul(sc[:], rinv[:], gp[:, gcol : gcol + nch])
                tb = scr.tile([P, nch], F32, tag="scr")
                nc.vector.tensor_mul(tb[:], mean, sc[:])
                bi = affp.tile([P, nch], F32, tag="aff")
                nc.vector.tensor_sub(bi[:], gp[:, bcol : bcol + nch], tb[:])
                if not plus_one:
                    return sc, bi, None
                bip = affp.tile([P, nch], F32, tag="aff")
                nc.vector.tensor_scalar_add(bip[:], bi[:], 1.0)
                return sc, bi, bip

            def pe_warm(pool, n, name, tag):
                dp = pool.tile([P, 64], F32, tag=tag, name=name)
                for _ in range(n):
                    nc.tensor.matmul(
                        dp[:], ident[:, 0:P], ident[:, 0:64], start=True, stop=True
                    )

            # ---------------- Stage 1: z1 = unfold(m1) @ f + kb ----------------
            # fp32r matmuls, two 8-bank PSUM waves, contraction-outer order so
            # PE consumes f tiles in DMA arrival order.
            z1_sb = {}
            stats1 = [statp.tile([P, B_LOC, 6], F32, tag="st", name="st1") for _ in range(KC)]
            es_ps1 = ExitStack()
            ps1 = es_ps1.enter_context(tc.tile_pool(name="ps1", bufs=8, space="PSUM"))
            for wave in (0, 1):
                mcs = (2 * wave, 2 * wave + 1)
                ps = {}
                for b in range(B_LOC):
                    for mc in mcs:
                        ps[b, mc] = ps1.tile([P, L], F32, tag="ps", name=f"ps1_{b}_{mc}")
                for kc in range(KC):
                    for w in range(W):
                        first = kc == 0 and w == 0
                        last = kc == KC - 1 and w == W - 1
                        for b in range(B_LOC):
                            for mc in mcs:
                                nc.tensor.matmul(
                                    ps[b, mc][:],
                                    f_sb[w, kc][:, mc * P : (mc + 1) * P],
                                    m1_sb[b, kc][:, w : w + L],
                                    start=first,
                                    stop=last,
                                )
                for b in range(B_LOC):
                    for mc in mcs:
                        zt = z1p.tile([P, L], BF16, tag="z1")
                        nc.vector.tensor_tensor(
                            out=zt[:], in0=ps[b, mc][:], in1=kbt1_sb[mc][:], op=ALU.add
                        )
                        nc.vector.bn_stats(out=stats1[mc][:, b, :], in_=zt[:])
                        z1_sb[b, mc] = zt

            pe_warm(ps1, 250, "warm1", "ps")
            pack1 = packp.tile([P, KC, 2], F32, tag="g")
            stats_to_pack(stats1, pack1)
            g1 = allreduce(pack1, KC * 2)
            sc1, bi1, bip1 = affines(g1, KC, _G1, _B1, plus_one=True)

            # elu': l' = elu(y)+1 = max(y+1, min(exp(y), 1)), y = sc*z + bi
            l_sb = {}
            for b in range(B_LOC):
                for mc in range(KC):
                    z = z1_sb[b, mc]
                    e = ebuf.tile([P, L], BF16, tag="e")
                    nc.scalar.activation(
                        out=e[:], in_=z[:], func=AF.Exp,
                        bias=bi1[:, mc : mc + 1], scale=sc1[:, mc : mc + 1],
                    )
                    y1 = ebuf.tile([P, L], BF16, tag="e")
                    nc.vector.tensor_scalar(
                        out=y1[:], in0=z[:],
                        scalar1=sc1[:, mc : mc + 1], scalar2=bip1[:, mc : mc + 1],
                        op0=ALU.mult, op1=ALU.add,
                    )
                    lt = lp.tile([P, L], BF16, tag="l")
                    nc.vector.scalar_tensor_tensor(
                        out=lt[:], in0=e[:], scalar=1.0, in1=y1[:],
                        op0=ALU.min, op1=ALU.max,
                    )
                    l_sb[b, mc] = lt

            es_ps1.close()
            es_B.close()

            # ---- stage-2..4-scoped pools
            es_C = ExitStack()
            z23p = es_C.enter_context(tc.tile_pool(name="z23", bufs=2 * B_LOC * KC))
            qkp = es_C.enter_context(tc.tile_pool(name="qk", bufs=2 * B_LOC * KC))
            es_psB = ExitStack()
            psB = es_psB.enter_context(tc.tile_pool(name="psB", bufs=4, space="PSUM"))
            psT = es_psB.enter_context(tc.tile_pool(name="psT", bufs=2, space="PSUM"))

            # ------------- Stage 2/3: q2 = l@wq + qb', k2 = l@wk + kb' -------------
            stats2 = [statp.tile([P, B_LOC, 6], F32, tag="st", name="st2") for _ in range(KC)]
            stats3 = [statp.tile([P, B_LOC, 6], F32, tag="st", name="st3") for _ in range(KC)]
            z2_sb, z3_sb = {}, {}
            for b in range(B_LOC):
                for mc in range(KC):
                    for w_sb, bias_sb, zdst, stats, move_eng in (
                        (wq_sb, qbt_sb, z2_sb, stats2, nc.vector),
                        (wk_sb, kbt3_sb, z3_sb, stats3, nc.vector),
                    ):
                        ps = psB.tile([P, L], F32, tag="ps")
                        for kc in range(KC):
                            nc.tensor.matmul(
                                ps[:],
                                w_sb[kc][:, mc * P : (mc + 1) * P],
                                l_sb[b, kc][:],
                                start=(kc == 0),
                                stop=(kc == KC - 1),
                            )
                        zt = z23p.tile([P, L], BF16, tag="z23")
                        move_eng.tensor_tensor(
                            out=zt[:], in0=ps[:], in1=bias_sb[mc][:], op=ALU.add
                        )
                        nc.vector.bn_stats(out=stats[mc][:, b, :], in_=zt[:])
                        zdst[b, mc] = zt

            pack23 = packp.tile([P, 2 * KC, 2], F32, tag="g")
            stats_to_pack(stats2 + stats3, pack23)
            g23 = allreduce(pack23, 4 * KC)
            sc2, bi2, _ = affines(g23[:, 0 : 2 * KC], KC, _G2, _B2)
            sc3, bi3, _ = affines(g23[:, 2 * KC : 4 * KC], KC, _G3, _B3)

            # transposes of l' for stage 5 fill the AR2/3 PE-idle window
            lstd_sb = {}
            for b in range(B_LOC):
                for kc in range(KC):
                    pst = psT.tile([P, L], BF16, tag="psT")
                    for mc in range(KC):
                        nc.tensor.transpose(
                            pst[:, mc * P : (mc + 1) * P],
                            l_sb[b, mc][:, kc * P : (kc + 1) * P],
                            ident[:],
                        )
                    lst = lsp.tile([P, E], BF16, tag="lstd")
                    nc.vector.tensor_scalar_add(lst[:], pst[:], -1.0)
                    lstd_sb[b, kc] = lst
            pe_warm(psT, 130, "warm23", "psT")

            # elu (exact): classic = Exp+Relu on Act then min/sub + add;
            # dve form = Exp on Act then affine + min/sub + max on DVE/Pool.
            def elu_exact(z, sc, bi, mc, out_pool, out_tag, out_dtype, style):
                e = ebuf.tile([P, L], BF16, tag="e")
                nc.scalar.activation(
                    out=e[:], in_=z[:], func=AF.Exp,
                    bias=bi[:, mc : mc + 1], scale=sc[:, mc : mc + 1],
                )
                o = out_pool.tile([P, L], out_dtype, tag=out_tag)
                if style == "act":
                    r = ebuf.tile([P, L], BF16, tag="e")
                    nc.scalar.activation(
                        out=r[:], in_=z[:], func=AF.Relu,
                        bias=bi[:, mc : mc + 1], scale=sc[:, mc : mc + 1],
                    )
                    t = ebuf.tile([P, L], BF16, tag="e")
                    nc.vector.tensor_scalar(
                        out=t[:], in0=e[:], scalar1=1.0, scalar2=1.0,
                        op0=ALU.min, op1=ALU.subtract,
                    )
                    nc.gpsimd.tensor_tensor(out=o[:], in0=r[:], in1=t[:], op=ALU.add)
                else:
                    y = ebuf.tile([P, L], BF16, tag="e")
                    nc.vector.tensor_scalar(
                        out=y[:], in0=z[:],
                        scalar1=sc[:, mc : mc + 1], scalar2=bi[:, mc : mc + 1],
                        op0=ALU.mult, op1=ALU.add,
                    )
                    t = ebuf.tile([P, L], BF16, tag="e")
                    nc.vector.tensor_scalar(
                        out=t[:], in0=e[:], scalar1=1.0, scalar2=0.0,
                        op0=ALU.subtract, op1=ALU.min,
                    )
                    nc.vector.tensor_tensor(out=o[:], in0=y[:], in1=t[:], op=ALU.max)
                return o

            q2_sb, k2_sb = {}, {}
            for b in range(B_LOC):
                for mc in range(KC):
                    q2_sb[b, mc] = elu_exact(
                        z2_sb[b, mc], sc2, bi2, mc, qkp, "qk", BF16, "act"
                    )
                    k2_sb[b, mc] = elu_exact(
                        z3_sb[b, mc], sc3, bi3, mc, qkp, "qk", BF16, "dve"
                    )

            # ------------- Stage 4: wT = (q2 @ k2^T)^T + wb^T, BN4, softmax -------------
            stats4 = [statp.tile([P, B_LOC, 6], F32, tag="st", name="st4") for _ in range(KC)]
            wt_sb, mx_sb = {}, {}
            for b in range(B_LOC):
                for kc in range(KC):
                    ps = psB.tile([P, L], F32, tag="ps")
                    for ec in range(KC):
                        nc.tensor.matmul(
                            ps[:],
                            k2_sb[b, ec][:, kc * P : (kc + 1) * P],
                            q2_sb[b, ec][:],
                            start=(ec == 0),
                            stop=(ec == KC - 1),
                        )
                    wt = wtp.tile([P, L], BF16, tag="wT")
                    nc.vector.tensor_tensor(out=wt[:], in0=ps[:], in1=wbt_sb[kc][:], op=ALU.add)
                    nc.vector.bn_stats(out=stats4[kc][:, b, :], in_=wt[:])
                    mx = mxp.tile([P, 1], F32, tag="mx")
                    nc.vector.tensor_reduce(out=mx[:], in_=wt[:], axis=AX.X, op=ALU.max)
                    wt_sb[b, kc] = wt
                    mx_sb[b, kc] = mx

            pe_warm(psT, 130, "warm4", "psT")
            pack4 = packp.tile([P, KC, 2], F32, tag="g")
            stats_to_pack(stats4, pack4)
            g4 = allreduce(pack4, KC * 2)
            sc4, bi4, _ = affines(g4, KC, _G4, _B4)

            # softmax over q (free axis): exp(sc4*x - sc4*mx), scale by 1/sum
            for b in range(B_LOC):
                for kc in range(KC):
                    wt = wt_sb[b, kc]
                    eb = scr.tile([P, 1], F32, tag="scr")
                    nc.vector.tensor_scalar(
                        out=eb[:], in0=mx_sb[b, kc][:],
                        scalar1=sc4[:, kc : kc + 1], scalar2=-1.0,
                        op0=ALU.mult, op1=ALU.mult,
                    )
                    ssum = scr.tile([P, 1], F32, tag="scr")
                    nc.scalar.activation(
                        out=wt[:], in_=wt[:], func=AF.Exp,
                        bias=eb[:], scale=sc4[:, kc : kc + 1], accum_out=ssum[:],
                    )
                    rs = scr.tile([P, 1], F32, tag="scr")
                    nc.vector.reciprocal(rs[:], ssum[:])
                    nc.vector.tensor_scalar_mul(wt[:], wt[:], rs[:])

            es_C.close()

            # ---------------- Stage 5: out = w @ l, BN5 + ELU ----------------
            es_D = ExitStack()
            z5p = es_D.enter_context(tc.tile_pool(name="z5", bufs=B_LOC * KC))
            outp = es_D.enter_context(tc.tile_pool(name="out", bufs=B_LOC * KC))
            stats5 = [statp.tile([P, B_LOC, 6], F32, tag="st", name="st5") for _ in range(KC)]
            z5_sb = {}
            for b in range(B_LOC):
                for mc in range(KC):
                    ps = psB.tile([P, L], F32, tag="ps")
                    for kc in range(KC):
                        nc.tensor.matmul(
                            ps[:],
                            lstd_sb[b, kc][:, mc * P : (mc + 1) * P],
                            wt_sb[b, kc][:],
                            start=(kc == 0),
                            stop=(kc == KC - 1),
                        )
                    zt = z5p.tile([P, L], BF16, tag="z5")
                    nc.scalar.activation(out=zt[:], in_=ps[:], func=AF.Copy)
                    nc.vector.bn_stats(out=stats5[mc][:, b, :], in_=zt[:])
                    z5_sb[b, mc] = zt

            pack5 = packp.tile([P, KC, 2], F32, tag="g")
            stats_to_pack(stats5, pack5)
            g5 = allreduce(pack5, KC * 2)
            sc5, bi5, _ = affines(g5, KC, _G5, _B5)

            for b in range(B_LOC):
                for mc in range(KC):
                    style = "act" if (b * KC + mc) % 2 == 0 else "dve"
                    ot = elu_exact(
                        z5_sb[b, mc], sc5, bi5, mc, outp, "out", F32, style
                    )
                    nc.sync.dma_start(
                        out=out_d[b, mc * P : (mc + 1) * P, :], in_=ot[:]
                    )

            es_D.close()
            es_psB.close()

    _split_waits(nc)
    _CACHE["nc"] = nc
    return nc


def _pack_affine(vecs):
    cols = []
    for v in vecs:
        cols.append(np.ascontiguousarray(np.asarray(v, np.float32).reshape(KC, P).T))
    return np.ascontiguousarray(np.concatenate(cols, axis=1))


def kernel(m1, f, wq, wk, qb, kb, wb, g1, b1, g2, b2, g3, b3, g4, b4, g5, b5):
    m1 = np.asarray(m1, np.float32)
    nc = _build()
    m1t_ = m1.transpose(0, 2, 1)
    m1t = np.zeros((B, E, L + 2 * S), np.float32)
    m1t[:, :, S : S + L] = m1t_
    f_h = np.ascontiguousarray(np.asarray(f, np.float32))
    wq_bf = np.asarray(wq, np.float32).astype(ml_dtypes.bfloat16)
    wk_bf = np.asarray(wk, np.float32).astype(ml_dtypes.bfloat16)
    # stage-1 output is l' = elu+1; fold the -1 into the stage-2/3 biases
    # using the column sums of the bf16-rounded weights (what the HW adds).
    qb_c = np.asarray(qb, np.float32) - wq_bf.astype(np.float32).sum(axis=0)[None, :]
    kb_c = np.asarray(kb, np.float32) - wk_bf.astype(np.float32).sum(axis=0)[None, :]
    qbt = np.ascontiguousarray(qb_c.T)
    kbt1 = np.ascontiguousarray(np.asarray(kb, np.float32).T)
    kbt3 = np.ascontiguousarray(kb_c.T)
    wbt = np.ascontiguousarray(np.asarray(wb, np.float32).T)
    gpack = _pack_affine([g1, b1, g2, b2, g3, b3, g4, b4, g5, b5])

    shared = {
        "f": f_h, "wq": wq_bf, "wk": wk_bf,
        "qbt": qbt, "kbt1": kbt1, "kbt3": kbt3, "wbt": wbt, "gpack": gpack,
    }
    in_maps = [
        {"m1t": np.ascontiguousarray(m1t[i * B_LOC : (i + 1) * B_LOC]), **shared}
        for i in range(N_CORES)
    ]
    trace = os.environ.get("KERNEL_TRACE") == "1"
    res = run_bass_kernel_spmd(nc, in_maps, list(range(N_CORES)), trace=trace)
    _CACHE["last_results"] = res

    out = np.empty((B, L, E), np.float32)
    for i in range(N_CORES):
        out[i * B_LOC : (i + 1) * B_LOC] = res.results[i]["outt"].transpose(0, 2, 1)
    return out


# revision 20
# speedup vs baseline: 2.2672x; 1.0206x over previous
"""Bass/Tile TRN2 kernel for nn_LAN_4320737100678 (dense transformer block).

Data-parallel over the batch axis across 8 NeuronCores (4 batches/core).
Activations are feature-major ([E, L] per batch) so BatchNorm reductions
and the softmax run along the free axis. BN moments are globalized with
four in-kernel AllReduces (BN2+BN3 share a round) plus one warmup
AllReduce at t=0 that absorbs the CC-stream startup cost.

Perf structure vs the fp32 baseline:
 - stage-1 matmuls run in float32r (1 cyc/row vs 4 for fp32), stages 2-5
   and the l-transposes run in bf16.
 - stage-1 output is kept as l' = elu+1 (one Act pass + two vector
   passes); the -1 is folded into host-corrected stage-2/3 biases and
   into the transpose copy for stage 5.
 - rsqrt for the BN affines is Exp(-0.5*Ln(var+eps)) so every activation
   (Exp/Relu/Ln/Copy) lives in one act table -> no table reloads.
 - elementwise work is split across DVE / Pool / Act to keep each under
   the PE roofline.
"""

import os
import sys

sys.path.insert(0, "/opt/trn_rl_repo")

import ml_dtypes
import numpy as np

import concourse.bass as bass
import concourse.tile as tile
from concourse import mybir
from concourse.bass_utils import run_bass_kernel_spmd
from concourse.masks import make_identity

N_CORES = 8
B, L, E, W = 32, 512, 512, 5
S = W // 2
P = 128
KC = E // P            # feature chunks of 128
B_LOC = B // N_CORES   # batches per core
EPS = 1e-3
F32 = mybir.dt.float32
F32R = mybir.dt.float32r
BF16 = mybir.dt.bfloat16
AF = mybir.ActivationFunctionType
ALU = mybir.AluOpType
AX = mybir.AxisListType

# gpack column base offsets (each vector packed as [P, KC])
_G1, _B1, _G2, _B2, _G3, _B3, _G4, _B4, _G5, _B5 = (i * KC for i in range(10))

_MAX_CTRL_WAITS = 1


def _split_waits(nc, max_waits=_MAX_CTRL_WAITS):
    """walrus in this container encodes at most one sync-wait slot per
    instruction. Hoist extra waits onto same-engine NOPs inserted right
    before the owning instruction (same engine => executes first)."""
    for fn in nc.m.functions:
        for bb in fn.blocks:
            rebuilt = []
            changed = False
            for ins in bb.instructions:
                si = ins.sync_info
                if si is not None and len(si.on_wait) > max_waits:
                    waits = list(si.on_wait)
                    rest = waits[max_waits:]
                    for j in range(0, len(rest), max_waits):
                        nop = mybir.InstNoOp(
                            name=f"{ins.name}_wsplit{j}",
                            engine=ins.engine,
                            bass_nofuse=True,
                            sync_info=mybir.SyncInfo(
                                on_wait=rest[j : j + max_waits], on_update=[]
                            ),
                        )
                        rebuilt.append(nop)
                    ins.sync_info = mybir.SyncInfo(
                        on_wait=waits[:max_waits], on_update=list(si.on_update)
                    )
                    changed = True
                rebuilt.append(ins)
            if changed:
                bb.instructions = rebuilt


def _r(ap):
    return ap.bitcast(F32R)


_CACHE = {}


def _build():
    if "nc" in _CACHE:
        return _CACHE["nc"]
    from contextlib import ExitStack

    nc = bass.Bass("TRN2", target_bir_lowering=False, debug=False, num_devices=N_CORES)

    m1t_d = nc.dram_tensor("m1t", [B_LOC, E, L + 2 * S], F32R, kind="ExternalInput")
    f_d = nc.dram_tensor("f", [W * E, E], F32R, kind="ExternalInput")
    wq_d = nc.dram_tensor("wq", [E, E], BF16, kind="ExternalInput")
    wk_d = nc.dram_tensor("wk", [E, E], BF16, kind="ExternalInput")
    qbt_d = nc.dram_tensor("qbt", [E, L], F32, kind="ExternalInput")   # corrected
    kbt1_d = nc.dram_tensor("kbt1", [E, L], F32, kind="ExternalInput")  # exact kb^T
    kbt3_d = nc.dram_tensor("kbt3", [E, L], F32, kind="ExternalInput")  # corrected
    wbt_d = nc.dram_tensor("wbt", [L, L], F32, kind="ExternalInput")
    gp_d = nc.dram_tensor("gpack", [P, 10 * KC], F32, kind="ExternalInput")
    out_d = nc.dram_tensor("outt", [B_LOC, E, L], F32, kind="ExternalOutput")

    groups = [list(range(N_CORES))]

    with tile.TileContext(nc) as tc:
        with (
            tc.tile_pool(name="const", bufs=1) as const,
            tc.tile_pool(name="aff", bufs=16) as affp,
            tc.tile_pool(name="stats", bufs=24) as statp,
            tc.tile_pool(name="packs", bufs=8) as packp,
            tc.tile_pool(name="scr", bufs=24) as scr,
            tc.tile_pool(name="ebuf", bufs=6) as ebuf,
            tc.tile_pool(name="l", bufs=B_LOC * KC) as lp,
            tc.tile_pool(name="wqk", bufs=2 * KC) as wqkp,
            tc.tile_pool(name="wbt", bufs=KC) as wbtp,
            tc.tile_pool(name="qkb", bufs=2 * KC) as qk_biasp,
            tc.tile_pool(name="dram", bufs=12, space="DRAM") as dram,
        ):
            # ---- warmup AllReduce: absorbs CC-stream startup + syncs cores
            warm = const.tile([P, 2], F32, tag="warm")
            nc.vector.memset(warm[:], 0.0)
            cc_w_in = dram.tile([P, 2], F32, tag="cc")
            cc_w_out = dram.tile([P, 2], F32, tag="cc")
            nc.sync.dma_start(out=cc_w_in[:], in_=warm[:])
            nc.gpsimd.collective_compute(
                "AllReduce", ALU.add, replica_groups=groups,
                ins=[cc_w_in.opt()], outs=[cc_w_out.opt()],
            )

            # ---- constants
            ident = const.tile([P, P], BF16, tag="ident")
            make_identity(nc, ident[:])
            epst = const.tile([P, 1], F32, tag="eps")
            nc.vector.memset(epst[:], EPS)

            # ---- stage-1-scoped pools (LIFO: closed before stage-2 pools open)
            es_B = ExitStack()
            z1p = es_B.enter_context(tc.tile_pool(name="z1", bufs=B_LOC * KC))
            kb1p = es_B.enter_context(tc.tile_pool(name="kb1", bufs=KC))
            fp = es_B.enter_context(tc.tile_pool(name="f", bufs=W * KC))
            mp = es_B.enter_context(tc.tile_pool(name="m1", bufs=B_LOC * KC))

            # stage-1 inputs, interleaved by kc so PE can start early
            f_sb, m1_sb = {}, {}

            def load_f(w, kc):
                t = fp.tile([P, E], F32R, tag="f", name=f"f_{w}_{kc}")
                r0 = (w * KC + kc) * P
                nc.sync.dma_start(out=t[:], in_=f_d[r0 : r0 + P, :])
                f_sb[w, kc] = t

            def load_m1(b, kc):
                t = mp.tile([P, L + 2 * S], F32R, tag="m1", name=f"m1_{b}_{kc}")
                nc.sync.dma_start(out=t[:], in_=m1t_d[b, kc * P : (kc + 1) * P, :])
                m1_sb[b, kc] = t

            for kc in range(KC):
                load_f(0, kc)
                load_m1(0, kc)
                for b in range(1, B_LOC):
                    load_m1(b, kc)
                for w in range(1, W):
                    load_f(w, kc)

            # weights / biases that can trickle in during stage 1 (Act queue)
            wq_sb, wk_sb, kbt1_sb, wbt_sb, qbt_sb, kbt3_sb = {}, {}, {}, {}, {}, {}
            for kc in range(KC):
                t = wqkp.tile([P, E], BF16, tag="wqk")
                nc.scalar.dma_start(out=t[:], in_=wq_d[kc * P : (kc + 1) * P, :])
                wq_sb[kc] = t
                t = wqkp.tile([P, E], BF16, tag="wqk")
                nc.scalar.dma_start(out=t[:], in_=wk_d[kc * P : (kc + 1) * P, :])
                wk_sb[kc] = t
                t = kb1p.tile([P, L], F32, tag="kb1")
                nc.scalar.dma_start(out=t[:], in_=kbt1_d[kc * P : (kc + 1) * P, :])
                kbt1_sb[kc] = t
            for kc in range(KC):
                t = wbtp.tile([P, L], F32, tag="wbt")
                nc.scalar.dma_start(out=t[:], in_=wbt_d[kc * P : (kc + 1) * P, :])
                wbt_sb[kc] = t
                t = qk_biasp.tile([P, L], F32, tag="qkb")
                nc.scalar.dma_start(out=t[:], in_=qbt_d[kc * P : (kc + 1) * P, :])
                qbt_sb[kc] = t
                t = qk_biasp.tile([P, L], F32, tag="qkb")
                nc.scalar.dma_start(out=t[:], in_=kbt3_d[kc * P : (kc + 1) * P, :])
                kbt3_sb[kc] = t
            gp = const.tile([P, 10 * KC], F32, tag="gp")
            nc.scalar.dma_start(out=gp[:], in_=gp_d[:])

            # ---------------- helpers ----------------
            def allreduce(pack, width):
                cc_in = dram.tile([P, width], F32, tag="cc")
                cc_out = dram.tile([P, width], F32, tag="cc")
                nc.sync.dma_start(out=cc_in[:], in_=pack[:])
                nc.gpsimd.collective_compute(
                    "AllReduce", ALU.add, replica_groups=groups,
                    ins=[cc_in.opt()], outs=[cc_out.opt()],
                )
                g = packp.tile([P, width], F32, tag="g")
                nc.sync.dma_start(out=g[:], in_=cc_out[:])
                return g

            def stats_to_pack(stats_tiles, pack):
                """stats_tiles: per-chunk [P, B_LOC, 6] bn_stats rows.
                pack[:, c, 0] = local_mean/8, pack[:, c, 1] = local_E[x^2]/8."""
                n = len(stats_tiles)
                for c, st in enumerate(stats_tiles):
                    nc.vector.bn_aggr(out=pack[:, c, :], in_=st[:])
                mean = pack[:, :, 0]
                ex2 = pack[:, :, 1]
                msq = scr.tile([P, n], F32, tag="scr")
                nc.vector.tensor_mul(msq[:], mean, mean)
                nc.vector.tensor_add(ex2, ex2, msq[:])
                nc.vector.tensor_scalar_mul(pack[:], pack[:], 1.0 / N_CORES)

            def affines(g, nch, gcol, bcol, plus_one=False):
                """From allreduced [P, nch, 2] (mean, E[x^2]) compute [P, nch]
                scale = gamma*rsqrt(var+eps), bias = beta - mean*scale.
                rsqrt = Exp(-0.5*Ln(var+eps)) (stays in the exp act table)."""
                gv = g.rearrange("p (c two) -> p c two", two=2)
                mean = gv[:, :, 0]
                ex2 = gv[:, :, 1]
                msq = scr.tile([P, nch], F32, tag="scr")
                nc.vector.tensor_mul(msq[:], mean, mean)
                var = scr.tile([P, nch], F32, tag="scr")
                nc.vector.tensor_sub(var[:], ex2, msq[:])
                lnv = scr.tile([P, nch], F32, tag="scr")
                nc.scalar.activation(out=lnv[:], in_=var[:], func=AF.Ln, bias=epst[:])
                rinv = scr.tile([P, nch], F32, tag="scr")
                nc.scalar.activation(out=rinv[:], in_=lnv[:], func=AF.Exp, scale=-0.5)
                sc = affp.tile([P, nch], F32, tag="aff")
                nc.vector.tensor_mul(sc[:], rinv[:], gp[:, gcol : gcol + nch])
                tb = scr.tile([P, nch], F32, tag="scr")
                nc.vector.tensor_mul(tb[:], mean, sc[:])
                bi = affp.tile([P, nch], F32, tag="aff")
                nc.vector.tensor_sub(bi[:], gp[:, bcol : bcol + nch], tb[:])
                if not plus_one:
                    return sc, bi, None
                bip = affp.tile([P, nch], F32, tag="aff")
                nc.vector.tensor_scalar_add(bip[:], bi[:], 1.0)
                return sc, bi, bip

            def pe_warm(pool, n, name, tag):
                dp = pool.tile([P, 64], F32, tag=tag, name=name)
                for _ in range(n):
                    nc.tensor.matmul(
                        dp[:], ident[:, 0:P], ident[:, 0:64], start=True, stop=True
                    )

            def pe_warm_after(pool, g, n, name, tag):
                """Warm-up matmuls gated on the AllReduce result g: the first
                one reads g (bitcast to bf16) so the chain starts right when
                the AR lands, re-ramping the PE while affines/elu run."""
                dp = pool.tile([P, 64], F32, tag=tag, name=name)
                gb = g[:].bitcast(BF16)
                nw = gb.free_size()
                nc.tensor.matmul(
                    dp[:, 0:nw], ident[:, 0:P], gb, start=True, stop=True
                )
                for _ in range(n - 1):
                    nc.tensor.matmul(
                        dp[:], ident[:, 0:P], ident[:, 0:64], start=True, stop=True
                    )

            # ---------------- Stage 1: z1 = unfold(m1) @ f + kb ----------------
            # fp32r matmuls, two 8-bank PSUM waves, contraction-outer order so
            # PE consumes f tiles in DMA arrival order.
            z1_sb = {}
            stats1 = [statp.tile([P, B_LOC, 6], F32, tag="st", name="st1") for _ in range(KC)]
            es_ps1 = ExitStack()
            ps1 = es_ps1.enter_context(tc.tile_pool(name="ps1", bufs=8, space="PSUM"))
            for wave in (0, 1):
                mcs = (2 * wave, 2 * wave + 1)
                ps = {}
                for b in range(B_LOC):
                    for mc in mcs:
                        ps[b, mc] = ps1.tile([P, L], F32, tag="ps", name=f"ps1_{b}_{mc}")
                for kc in range(KC):
                    for w in range(W):
                        first = kc == 0 and w == 0
                        last = kc == KC - 1 and w == W - 1
                        for b in range(B_LOC):
                            for mc in mcs:
                                nc.tensor.matmul(
                                    ps[b, mc][:],
                                    f_sb[w, kc][:, mc * P : (mc + 1) * P],
                                    m1_sb[b, kc][:, w : w + L],
                                    start=first,
                                    stop=last,
                                )
                for b in range(B_LOC):
                    for mc in mcs:
                        zt = z1p.tile([P, L], BF16, tag="z1")
                        nc.vector.tensor_tensor(
                            out=zt[:], in0=ps[b, mc][:], in1=kbt1_sb[mc][:], op=ALU.add
                        )
                        nc.vector.bn_stats(out=stats1[mc][:, b, :], in_=zt[:])
                        z1_sb[b, mc] = zt

            pe_warm(ps1, 150, "warm1", "ps")
            pack1 = packp.tile([P, KC, 2], F32, tag="g")
            stats_to_pack(stats1, pack1)
            g1 = allreduce(pack1, KC * 2)
            pe_warm_after(ps1, g1, 24, "warmg1", "ps")
            sc1, bi1, bip1 = affines(g1, KC, _G1, _B1, plus_one=True)

            # elu': l' = elu(y)+1, alternating between an Act-heavy form
            # (r + min(e,1)) and a DVE-heavy form (max(y+1, min(e,1))) so the
            # post-AR burst is spread across engines.
            l_sb = {}
            for b in range(B_LOC):
                for mc in range(KC):
                    z = z1_sb[b, mc]
                    e = ebuf.tile([P, L], BF16, tag="e")
                    nc.scalar.activation(
                        out=e[:], in_=z[:], func=AF.Exp,
                        bias=bi1[:, mc : mc + 1], scale=sc1[:, mc : mc + 1],
                    )
                    lt = lp.tile([P, L], BF16, tag="l")
                    if (b * KC + mc) % 2 == 0:
                        r = ebuf.tile([P, L], BF16, tag="e")
                        nc.scalar.activation(
                            out=r[:], in_=z[:], func=AF.Relu,
                            bias=bi1[:, mc : mc + 1], scale=sc1[:, mc : mc + 1],
                        )
                        m = ebuf.tile([P, L], BF16, tag="e")
                        nc.vector.tensor_scalar_min(m[:], e[:], 1.0)
                        nc.gpsimd.tensor_tensor(out=lt[:], in0=r[:], in1=m[:], op=ALU.add)
                    else:
                        y1 = ebuf.tile([P, L], BF16, tag="e")
                        nc.vector.tensor_scalar(
                            out=y1[:], in0=z[:],
                            scalar1=sc1[:, mc : mc + 1], scalar2=bip1[:, mc : mc + 1],
                            op0=ALU.mult, op1=ALU.add,
                        )
                        nc.vector.scalar_tensor_tensor(
                            out=lt[:], in0=e[:], scalar=1.0, in1=y1[:],
                            op0=ALU.min, op1=ALU.max,
                        )
                    l_sb[b, mc] = lt

            es_ps1.close()
            es_B.close()

            # ---- stage-2..5-scoped pools
            es_C0 = ExitStack()
            wtp = es_C0.enter_context(tc.tile_pool(name="wT", bufs=B_LOC * KC))
            wtbp = es_C0.enter_context(tc.tile_pool(name="wTb", bufs=B_LOC * KC))
            lsp = es_C0.enter_context(tc.tile_pool(name="lstd", bufs=B_LOC * KC))
            es_C = ExitStack()
            z23p = es_C.enter_context(tc.tile_pool(name="z23", bufs=2 * B_LOC * KC))
            qkp = es_C.enter_context(tc.tile_pool(name="qk", bufs=2 * B_LOC * KC))
            es_psB = ExitStack()
            psB = es_psB.enter_context(tc.tile_pool(name="psB", bufs=4, space="PSUM"))
            psT = es_psB.enter_context(tc.tile_pool(name="psT", bufs=2, space="PSUM"))

            # ------------- Stage 2/3: q2 = l@wq + qb', k2 = l@wk + kb' -------------
            stats2 = [statp.tile([P, B_LOC, 6], F32, tag="st", name="st2") for _ in range(KC)]
            stats3 = [statp.tile([P, B_LOC, 6], F32, tag="st", name="st3") for _ in range(KC)]
            z2_sb, z3_sb = {}, {}
            for b in range(B_LOC):
                for mc in range(KC):
                    for w_sb, bias_sb, zdst, stats, move_eng in (
                        (wq_sb, qbt_sb, z2_sb, stats2, nc.vector),
                        (wk_sb, kbt3_sb, z3_sb, stats3, nc.vector),
                    ):
                        ps = psB.tile([P, L], F32, tag="ps")
                        for kc in range(KC):
                            nc.tensor.matmul(
                                ps[:],
                                w_sb[kc][:, mc * P : (mc + 1) * P],
                                l_sb[b, kc][:],
                                start=(kc == 0),
                                stop=(kc == KC - 1),
                            )
                        zt = z23p.tile([P, L], BF16, tag="z23")
                        move_eng.tensor_tensor(
                            out=zt[:], in0=ps[:], in1=bias_sb[mc][:], op=ALU.add
                        )
                        nc.vector.bn_stats(out=stats[mc][:, b, :], in_=zt[:])
                        zdst[b, mc] = zt

            pack23 = packp.tile([P, 2 * KC, 2], F32, tag="g")
            stats_to_pack(stats2 + stats3, pack23)
            g23 = allreduce(pack23, 4 * KC)
            pe_warm_after(psT, g23, 24, "warmg23", "psT")
            sc2, bi2, _ = affines(g23[:, 0 : 2 * KC], KC, _G2, _B2)
            sc3, bi3, _ = affines(g23[:, 2 * KC : 4 * KC], KC, _G3, _B3)

            # transposes of l' for stage 5 fill the AR2/3 PE-idle window
            lstd_sb = {}
            for b in range(B_LOC):
                for kc in range(KC):
                    pst = psT.tile([P, L], BF16, tag="psT")
                    for mc in range(KC):
                        nc.tensor.transpose(
                            pst[:, mc * P : (mc + 1) * P],
                            l_sb[b, mc][:, kc * P : (kc + 1) * P],
                            ident[:],
                        )
                    lst = lsp.tile([P, E], BF16, tag="lstd")
                    nc.vector.tensor_scalar_add(lst[:], pst[:], -1.0)
                    lstd_sb[b, kc] = lst
            pe_warm(psT, 100, "warm23", "psT")

            # elu (exact): classic = Exp+Relu on Act then min/sub + add;
            # dve form = Exp on Act then affine + min/sub + max on DVE/Pool.
            def elu_exact(z, sc, bi, mc, out_pool, out_tag, out_dtype, style):
                e = ebuf.tile([P, L], BF16, tag="e")
                nc.scalar.activation(
                    out=e[:], in_=z[:], func=AF.Exp,
                    bias=bi[:, mc : mc + 1], scale=sc[:, mc : mc + 1],
                )
                o = out_pool.tile([P, L], out_dtype, tag=out_tag)
                if style == "act":
                    r = ebuf.tile([P, L], BF16, tag="e")
                    nc.scalar.activation(
                        out=r[:], in_=z[:], func=AF.Relu,
                        bias=bi[:, mc : mc + 1], scale=sc[:, mc : mc + 1],
                    )
                    t = ebuf.tile([P, L], BF16, tag="e")
                    nc.vector.tensor_scalar(
                        out=t[:], in0=e[:], scalar1=1.0, scalar2=1.0,
                        op0=ALU.min, op1=ALU.subtract,
                    )
                    nc.gpsimd.tensor_tensor(out=o[:], in0=r[:], in1=t[:], op=ALU.add)
                else:
                    y = ebuf.tile([P, L], BF16, tag="e")
                    nc.vector.tensor_scalar(
                        out=y[:], in0=z[:],
                        scalar1=sc[:, mc : mc + 1], scalar2=bi[:, mc : mc + 1],
                        op0=ALU.mult, op1=ALU.add,
                    )
                    t = ebuf.tile([P, L], BF16, tag="e")
                    nc.vector.tensor_scalar(
                        out=t[:], in0=e[:], scalar1=1.0, scalar2=0.0,
                        op0=ALU.subtract, op1=ALU.min,
                    )
                    nc.vector.tensor_tensor(out=o[:], in0=y[:], in1=t[:], op=ALU.max)
                return o

            q2_sb, k2_sb = {}, {}
            for b in range(B_LOC):
                for mc in range(KC):
                    q2_sb[b, mc] = elu_exact(
                        z2_sb[b, mc], sc2, bi2, mc, qkp, "qk", BF16, "act"
                    )
                    k2_sb[b, mc] = elu_exact(
                        z3_sb[b, mc], sc3, bi3, mc, qkp, "qk", BF16, "dve"
                    )

            # ------------- Stage 4: wT = (q2 @ k2^T)^T + wb^T, BN4, softmax -------------
            stats4 = [statp.tile([P, B_LOC, 6], F32, tag="st", name="st4") for _ in range(KC)]
            wt_sb = {}
            for b in range(B_LOC):
                for kc in range(KC):
                    ps = psB.tile([P, L], F32, tag="ps")
                    for ec in range(KC):
                        nc.tensor.matmul(
                            ps[:],
                            k2_sb[b, ec][:, kc * P : (kc + 1) * P],
                            q2_sb[b, ec][:],
                            start=(ec == 0),
                            stop=(ec == KC - 1),
                        )
                    wt = wtp.tile([P, L], F32, tag="wT")
                    nc.vector.tensor_tensor(out=wt[:], in0=ps[:], in1=wbt_sb[kc][:], op=ALU.add)
                    nc.vector.bn_stats(out=stats4[kc][:, b, :], in_=wt[:])
                    wt_sb[b, kc] = wt

            pe_warm(psT, 100, "warm4", "psT")
            pack4 = packp.tile([P, KC, 2], F32, tag="g")
            stats_to_pack(stats4, pack4)
            g4 = allreduce(pack4, KC * 2)
            pe_warm_after(psT, g4, 20, "warmg4", "psT")
            sc4, bi4, _ = affines(g4, KC, _G4, _B4)

            # softmax over q (free axis): BN4-normalized logits are small, so
            # exp needs no max subtraction (softmax is shift-invariant; fp32
            # range is ample). exp output goes to bf16 for the stage-5 matmul.
            wtb_sb = {}
            for b in range(B_LOC):
                for kc in range(KC):
                    ssum = scr.tile([P, 1], F32, tag="scr")
                    wtb = wtbp.tile([P, L], BF16, tag="wTb")
                    nc.scalar.activation(
                        out=wtb[:], in_=wt_sb[b, kc][:], func=AF.Exp,
                        bias=bi4[:, kc : kc + 1], scale=sc4[:, kc : kc + 1],
                        accum_out=ssum[:],
                    )
                    rs = scr.tile([P, 1], F32, tag="scr")
                    nc.vector.reciprocal(rs[:], ssum[:])
                    nc.vector.tensor_scalar_mul(wtb[:], wtb[:], rs[:])
                    wtb_sb[b, kc] = wtb

            es_C.close()

            # ---------------- Stage 5: out = w @ l, BN5 + ELU ----------------
            es_D = ExitStack()
            z5p = es_D.enter_context(tc.tile_pool(name="z5", bufs=B_LOC * KC))
            outp = es_D.enter_context(tc.tile_pool(name="out", bufs=B_LOC * KC))
            stats5 = [statp.tile([P, B_LOC, 6], F32, tag="st", name="st5") for _ in range(KC)]
            z5_sb = {}
            for b in range(B_LOC):
                for mc in range(KC):
                    ps = psB.tile([P, L], F32, tag="ps")
                    for kc in range(KC):
                        nc.tensor.matmul(
                            ps[:],
                            lstd_sb[b, kc][:, mc * P : (mc + 1) * P],
                            wtb_sb[b, kc][:],
                            start=(kc == 0),
                            stop=(kc == KC - 1),
                        )
                    zt = z5p.tile([P, L], BF16, tag="z5")
                    nc.scalar.activation(out=zt[:], in_=ps[:], func=AF.Copy)
                    nc.vector.bn_stats(out=stats5[mc][:, b, :], in_=zt[:])
                    z5_sb[b, mc] = zt

            pack5 = packp.tile([P, KC, 2], F32, tag="g")
            stats_to_pack(stats5, pack5)
            g5 = allreduce(pack5, KC * 2)
            sc5, bi5, _ = affines(g5, KC, _G5, _B5)

            for b in range(B_LOC):
                for mc in range(KC):
                    style = "act" if (b * KC + mc) % 2 == 0 else "dve"
                    ot = elu_exact(
                        z5_sb[b, mc], sc5, bi5, mc, outp, "out", F32, style
                    )
                    nc.sync.dma_start(
                        out=out_d[b, mc * P : (mc + 1) * P, :], in_=ot[:]
                    )

            es_D.close()
            es_C0.close()
            es_psB.close()

    _split_waits(nc)
    _CACHE["nc"] = nc
    return nc


def _pack_affine(vecs):
    cols = []
    for v in vecs:
        cols.append(np.ascontiguousarray(np.asarray(v, np.float32).reshape(KC, P).T))
    return np.ascontiguousarray(np.concatenate(cols, axis=1))


def kernel(m1, f, wq, wk, qb, kb, wb, g1, b1, g2, b2, g3, b3, g4, b4, g5, b5):
    m1 = np.asarray(m1, np.float32)
    nc = _build()
    m1t_ = m1.transpose(0, 2, 1)
    m1t = np.zeros((B, E, L + 2 * S), np.float32)
    m1t[:, :, S : S + L] = m1t_
    f_h = np.ascontiguousarray(np.asarray(f, np.float32))
    wq_bf = np.asarray(wq, np.float32).astype(ml_dtypes.bfloat16)
    wk_bf = np.asarray(wk, np.float32).astype(ml_dtypes.bfloat16)
    # stage-1 output is l' = elu+1; fold the -1 into the stage-2/3 biases
    # using the column sums of the bf16-rounded weights (what the HW adds).
    qb_c = np.asarray(qb, np.float32) - wq_bf.astype(np.float32).sum(axis=0)[None, :]
    kb_c = np.asarray(kb, np.float32) - wk_bf.astype(np.float32).sum(axis=0)[None, :]
    qbt = np.ascontiguousarray(qb_c.T)
    kbt1 = np.ascontiguousarray(np.asarray(kb, np.float32).T)
    kbt3 = np.ascontiguousarray(kb_c.T)
    wbt = np.ascontiguousarray(np.asarray(wb, np.float32).T)
    gpack = _pack_affine([g1, b1, g2, b2, g3, b3, g4, b4, g5, b5])

    shared = {
        "f": f_h, "wq": wq_bf, "wk": wk_bf,
        "qbt": qbt, "kbt1": kbt1, "kbt3": kbt3, "wbt": wbt, "gpack": gpack,
    }
    in_maps = [
        {"m1t": np.ascontiguousarray(m1t[i * B_LOC : (i + 1) * B_LOC]), **shared}
        for i in range(N_CORES)
    ]
    trace = os.environ.get("KERNEL_TRACE") == "1"
    res = run_bass_kernel_spmd(nc, in_maps, list(range(N_CORES)), trace=trace)
    _CACHE["last_results"] = res

    out = np.empty((B, L, E), np.float32)
    for i in range(N_CORES):
        out[i * B_LOC : (i + 1) * B_LOC] = res.results[i]["outt"].transpose(0, 2, 1)
    return out
